# revision 19
# baseline (speedup 1.0000x reference)
"""Trainium2 Bass/Tile kernel for AttnBlock:
GroupNorm(32) -> 1x1 conv q,k,v -> softmax attention over N=4096 tokens
-> 1x1 conv proj -> residual.

Sharding: 8 cores = 2 (batch) x 4 (query-token shards of N).  Each core gets
the full x of its batch plus its n-shard slice, and produces the [C, N/4]
output shard.  No collectives.

Architecture (v4):
- All heavy matmuls are fp8 MatmulPerfMode.DoubleRow: the full K=256
  contraction in one instruction at 0.5 cycles/output-column.  DR stationary
  operands need their 256 weight elements contiguous per partition; every
  lhsT is laid out [.., kt(2), 128].
- GroupNorm stats via a PE Gram-matrix over the m-major fp8 x copy
  (diag -> sum x^2, ones-matmul -> sum x), diag extracted by one DVE
  scalar_tensor_tensor+accum per c-tile.
- No k tensor: S^T = x^T g with g = a*(w1^T q) [C, NSH] -- the PSUM->SBUF
  drain is the n-shard-sized g (2K lanes) instead of the m-sized k (8K).
  The k bias is dropped exactly (softmax shift invariance); q keeps its
  effective bias.
- No v tensor: attention accumulates over x itself:
  AVx[n, c] = sum_m E[m, n] x[c, m] (moving operand = resident xTw8),
  plus denominator columns from a tiny ones matmul per ns.  After
  normalize + transpose, ONE DoubleRow projection by w238 = a*(w3 w2)^T
  (host-folded w3@w2, scaled 2^19 for fp8) produces the output; the scale
  is undone in the final scalar_tensor_tensor against xqb = x + b3eff.
- Softmax over 2-bank [128,1024] S^T psum tiles; exp ns-subtiles split
  between ACT (true Exp -> fp8e5) and DVE (Schraudolph bits =
  round(logit*4*log2e + 60) as uint8 == fp8e5m2; e5m2 because logits span
  +-8).  Output APs are permuted so E tiles come out [ns, kt, j] -- the
  DR lhsT layout for AVx.
"""

import ml_dtypes
import numpy as np

import concourse.bacc as bacc
import concourse.mybir as mybir
import concourse.tile as tile
from concourse import bass_utils

f32 = mybir.dt.float32
bf16 = mybir.dt.bfloat16
fp8 = mybir.dt.float8e4
fp8e5 = mybir.dt.float8e5
u8 = mybir.dt.uint8
AF = mybir.ActivationFunctionType
ALU = mybir.AluOpType
DR = mybir.MatmulPerfMode.DoubleRow

B = 2
C = 256
N = 4096          # 16**3 tokens
NSH = N // 4      # 1024 tokens per core
G = 32
CPG = C // G      # channels per group
NPG = CPG * N     # elements per group
EPS = 1e-6
SCALE = C ** -0.5          # 1/16
LOG2E = float(1.0 / np.log(2.0))
WS = 524288.0              # 2^19 fp8-range scale on w23; undone in the stt
MT = N // 128              # 32 m-tiles

NCORES = 8

# cpack column layout
CV0, CV1 = 0, 8            # cvec slice0/1: [gamma, beta, b0, bout, -gamma]
GMA, GMB = 16, 48          # gmask per slice [128, 32] (1/NPG folded)
GTA, GTB = 80, 208         # gmaskT per slice [32, 128] on partitions 0:32
MZL = 336                  # zeros[128] | ident[128] | zeros[128]
IDT = 464
EPC = 720                  # eps column
IDW = 728                  # identity * WS (residual preload)
CPW = 856

# engine splits (True -> ACT, False -> DVE)
EXP_SPLIT = [2] * 32       # of 4 ns-subtiles per (half*16+pair), how many ACT
GEP_ACT = [True, False]    # g drain per c-slice
ON_ACT = [True, False, True, False] * 2  # normalize per (half*4 + ns)
TRP_ACT = [True, False, True, False]  # attnx drain per (half*2 + t)

N_WARMUP = 42
PHASE = 4


def _build_body(nc, tc, d):
    from contextlib import ExitStack

    ctx = ExitStack()
    pc = ctx.enter_context(tc.tile_pool(name="const", bufs=1))
    pb = ctx.enter_context(tc.tile_pool(name="big", bufs=1))
    pw = ctx.enter_context(tc.tile_pool(name="work", bufs=3))
    ptiny = ctx.enter_context(tc.tile_pool(name="tiny", bufs=2))
    # PSUM: sp = 2 x [128,1024] (2 banks each), ot = 4 x [128,512] (1 bank)
    sp = ctx.enter_context(tc.tile_pool(name="sp", bufs=2, space="PSUM"))
    ot = ctx.enter_context(tc.tile_pool(name="pot", bufs=4, space="PSUM"))

    # ---- tiny consts ----
    zcol = pc.tile([128, 1], f32, tag="zcol", name="zcol")
    nc.vector.memset(zcol[:], 0.0)
    nc.const_aps.aps[(f32, 0.0)] = zcol[:]
    ones4 = pc.tile([128, 2, 1], fp8, tag="ones4", name="ones4")
    nc.vector.memset(ones4[:], 1.0)
    ones5 = pc.tile([128, 2, 1], fp8e5, tag="ones5", name="ones5")
    nc.vector.memset(ones5[:], 1.0)

    # ---- PE warmup: dep-free matmuls bridge the DMA head + pstate ramp
    wdum = pc.tile([128, 128], bf16, tag="wdum", name="wdum")
    nc.vector.memset(wdum[:], 1.0)
    wslot = ot.tile([128, 512], f32, tag="warm", name="warm")
    for i in range(N_WARMUP):
        nc.tensor.matmul(wslot[:, 0:128], wdum[:], wdum[:],
                         start=True, stop=True)

    # ---- input DMAs: the DMA fabric is serial -- order by need.
    # xTw8[p, pr, t, kt, j] = x[t*128+j, (2*pr+kt)*128+p], in quarters
    xTw8 = pb.tile([128, 16, 2, 2, 128], fp8, tag="xTw8", name="xTw8")
    xTw8f = xTw8[:].rearrange("p a b c e -> p (a b c e)")
    qs = [nc.sync, nc.scalar]
    cpack = pc.tile([128, CPW], f32, tag="cpack", name="cpack")
    for qr in range(4):
        qs[qr % 2].dma_start(xTw8f[:, qr * 2048:(qr + 1) * 2048],
                             d["xTw8"][:, qr * 2048:(qr + 1) * 2048])
        if qr == 1:
            nc.sync.dma_start(cpack[:], d["cpack"][:])
        if qr == 2:
            xq8 = pb.tile([128, 2, NSH], fp8, tag="xq8", name="xq8")
            nc.scalar.dma_start(xq8[:].rearrange("p a b -> p (a b)"),
                                d["xq8"][:])
    # wb[p, oh, kt, j] = w0^T[kt*128+p, oh*128+j]
    wb = pb.tile([128, 2, 2, 128], bf16, tag="wb", name="wb")
    nc.sync.dma_start(wb[:].rearrange("p a b c -> p (a b c)"), d["wb"][:])
    # w1p8[p, cs, kto, j] = w1[kto*128+p, cs*128+j]  (plain w1, fp8)
    w1p8 = pb.tile([128, 2, 2, 128], fp8, tag="w1p8", name="w1p8")
    nc.scalar.dma_start(w1p8[:].rearrange("p a b c -> p (a b c)"), d["w1p8"][:])
    # w23t[p, oh, kt, j] = (w3 w2)^T[kt*128+p, oh*128+j]
    w23t = pb.tile([128, 2, 2, 128], bf16, tag="w23t", name="w23t")
    nc.sync.dma_start(w23t[:].rearrange("p a b c -> p (a b c)"), d["w23t"][:])
    # xs8[p, mt, kt, j] = x[kt*128+p, mt*128+j], halves
    xs8 = pb.tile([128, MT, 2, 128], fp8, tag="xs8", name="xs8")
    xs8f = xs8[:].rearrange("p a b c -> p (a b c)")
    nc.scalar.dma_start(xs8f[:, 0:4096], d["xs8"][:, 0:4096])
    nc.sync.dma_start(xs8f[:, 4096:8192], d["xs8"][:, 4096:8192])
    # xq (f32 residual) is emitted LAST -- only needed by the final stt

    cvec = [cpack[:, CV0:CV0 + 8], cpack[:, CV1:CV1 + 8]]
    gm = [cpack[:, GMA:GMA + 32], cpack[:, GMB:GMB + 32]]
    gmt = [cpack[0:32, GTA:GTA + 128], cpack[0:32, GTB:GTB + 128]]
    ident = cpack[:, IDT:IDT + 128]
    dmask = [cpack[:, IDT:IDT + 256], cpack[:, MZL:MZL + 256]]
    identw = cpack[:, IDW:IDW + 128]
    epscol = cpack[0:32, EPC:EPC + 1]

    # ---- GroupNorm stats via PE Gram over xTw8 ----
    # gtile: [t0 gram 0:256 | t0 sum-x 256 | pad | t1 gram 512:768 | t1 sum-x]
    gtile = sp.tile([128, 1024], f32, tag="sp", name="gram")
    goff = [0, 512]
    for pr in range(16):
        for t in range(2):
            lhs = xTw8[:, pr, t]
            nc.tensor.matmul(
                gtile[:, goff[t]:goff[t] + 256],
                lhs,
                xTw8[:, pr].rearrange("p t kt j -> p kt t j"),
                start=(pr == 0), stop=(pr == 15), perf_mode=DR,
            )
            nc.tensor.matmul(
                gtile[:, goff[t] + 256:goff[t] + 257],
                lhs, ones4[:],
                start=(pr == 0), stop=(pr == 15), perf_mode=DR,
            )
    # pt[t]: col0 = sum x, col1 = sum x^2 (diag extract)
    pt = [ptiny.tile([128, 2], f32, tag=f"pt{t}", name=f"pt{t}") for t in range(2)]
    trash = pw.tile([128, 256], f32, tag="trash", name="trash", bufs=2)
    for t in range(2):
        nc.vector.tensor_copy(pt[t][:, 0:1], gtile[:, goff[t] + 256:goff[t] + 257])
        nc.vector.scalar_tensor_tensor(
            trash[:], gtile[:, goff[t]:goff[t] + 256], 1.0, dmask[t],
            op0=ALU.mult, op1=ALU.mult, accum_out=pt[t][:, 1:2],
        )

    # group stats -> mu/rstd -> per-channel a, bfold
    s32 = ot.tile([128, 512], f32, tag="warm", name="s32")
    for t in range(2):
        nc.tensor.matmul(s32[0:32, 0:2], gm[t], pt[t][:],
                         start=(t == 0), stop=(t == 1))
    sg = ptiny.tile([32, 2], f32, tag="sg", name="sg")
    nc.vector.tensor_copy(sg[:], s32[0:32, 0:2])
    mr = ptiny.tile([32, 2], f32, tag="mr", name="mr")
    musq = ptiny.tile([32, 1], f32, tag="musq", name="musq")
    nc.vector.tensor_mul(musq[:], sg[:, 0:1], sg[:, 0:1])
    var = ptiny.tile([32, 1], f32, tag="var", name="var")
    nc.vector.tensor_sub(var[:], sg[:, 1:2], musq[:])
    std = ptiny.tile([32, 1], f32, tag="std", name="std")
    nc.scalar.activation(std[:], var[:], AF.Sqrt, bias=epscol)
    # dummy Exp preloads the exp table during the DMA head
    edum = ptiny.tile([128, 1], u8, tag="edum", name="edum")
    nc.scalar.activation(edum[:].bitcast(fp8e5), zcol[:], AF.Exp)
    nc.vector.reciprocal(mr[:, 1:2], std[:])
    nc.vector.tensor_mul(mr[:, 0:1], sg[:, 0:1], mr[:, 1:2])

    a_t, bfold_bf = [], []
    bc = ot.tile([128, 512], f32, tag="warm", name="bc")
    for t in range(2):
        nc.tensor.matmul(bc[:, 2 * t:2 * t + 2], gmt[t], mr[:],
                         start=True, stop=True)
    for t in range(2):
        a = pb.tile([128, 1], f32, tag=f"a{t}", name=f"a{t}")
        nc.vector.tensor_mul(a[:], bc[:, 2 * t + 1:2 * t + 2], cvec[t][:, 0:1])
        bf = pb.tile([128, 1], f32, tag=f"bf{t}", name=f"bf{t}")
        nc.vector.tensor_scalar(
            bf[:], bc[:, 2 * t:2 * t + 1], cvec[t][:, 4:5], cvec[t][:, 1:2],
            op0=ALU.mult, op1=ALU.add,
        )
        bb = pb.tile([128, 1], bf16, tag=f"bfb{t}", name=f"bfb{t}")
        nc.vector.tensor_copy(bb[:], bf[:])
        a_t.append(a)
        bfold_bf.append(bb)

    def _early_out(srcs):
        for oh in range(2):
            for ch in range(2):
                yt = pw.tile([128, 512], f32, tag="yt", name="yt")
                nc.vector.tensor_copy(
                    yt[:], srcs[oh][:, ch * 512:(ch + 1) * 512])
                nc.sync.dma_start(
                    d["y"][:, oh * NSH + ch * 512:oh * NSH + (ch + 1) * 512],
                    yt[:])

    # ---- effective biases (RAW weights -- emitted before the a-fold) ----
    beff0 = []
    for oh in range(2):
        bp = ot.tile([128, 512], f32, tag="warm", name="bp")
        for t in range(2):
            nc.tensor.matmul(bp[:, 0:1], wb[:, oh, t],
                             bfold_bf[t][:], start=(t == 0), stop=(t == 1))
        bs = pb.tile([128, 1], f32, tag=f"beff0_{oh}", name=f"beff0_{oh}")
        nc.scalar.activation(bs[:], bp[:, 0:1], AF.Identity,
                             bias=cvec[oh][:, 2:3])
        beff0.append(bs)
    b3eff = []
    for oh in range(2):
        bp = ot.tile([128, 512], f32, tag="warm", name="bp3")
        for t in range(2):
            nc.tensor.matmul(bp[:, 0:1], w23t[:, oh, t],
                             bfold_bf[t][:], start=(t == 0), stop=(t == 1))
        bs = pb.tile([128, 1], f32, tag=f"b3eff{oh}", name=f"b3eff{oh}")
        nc.scalar.activation(bs[:], bp[:, 0:1], AF.Identity,
                             bias=cvec[oh][:, 3:4])
        b3eff.append(bs)

    # xqb = x-shard + b3eff (f32); the final stt adds proj*2^-19 onto it.
    # xq is the last input DMA issued -- its transfer queues behind all the
    # early-needed inputs on the serial DMA fabric but lands well before the
    # first finish phase needs xqb.
    xq = pb.tile([128, 2, NSH], f32, tag="xq", name="xq")
    nc.sync.dma_start(xq[:].rearrange("p a b -> p (a b)"), d["xq"][:])
    xqb = pb.tile([128, 2, NSH], f32, tag="xqb", name="xqb")
    for t in range(2):
        nc.vector.tensor_scalar(xqb[:, t], xq[:, t], b3eff[t][:], None,
                                op0=ALU.add)

    # ---- fold a into the fp8 weights directly (one DVE op each) ----
    w018 = pb.tile([128, 2, 2, 128], fp8, tag="w018", name="w018")
    w238 = pb.tile([128, 2, 2, 128], fp8, tag="w238", name="w238")
    for t in range(2):
        nc.vector.tensor_scalar_mul(w018[:, :, t], wb[:, :, t], a_t[t][:])
        nc.vector.tensor_scalar(w238[:, :, t], w23t[:, :, t], a_t[t][:], WS,
                                op0=ALU.mult, op1=ALU.mult)

    if PHASE <= 1:
        _early_out([xq[:, 0], xq[:, 1]])
        ctx.close()
        return

    # ---- q = w0a @ xq + beff0 : fp8 [128, 2(kt=oh), NSH] ----
    q2 = pb.tile([128, 2, NSH], fp8, tag="q2", name="q2")
    for oh in range(2):
        qp = sp.tile([128, 1024], f32, tag="sp", name="qp")
        for ch in range(2):
            nc.tensor.matmul(
                qp[:, ch * 512:(ch + 1) * 512],
                w018[:, oh],
                xq8[:, :, ch * 512:(ch + 1) * 512],
                start=True, stop=True, perf_mode=DR,
            )
        for ch in range(2):
            dst = q2[:, oh, ch * 512:(ch + 1) * 512]
            qsl = qp[:, ch * 512:(ch + 1) * 512]
            if ch == 0:
                nc.scalar.activation(dst, qsl, AF.Identity,
                                     bias=beff0[oh][:])
            else:
                nc.vector.tensor_scalar(dst, qsl, beff0[oh][:], None,
                                        op0=ALU.add)

    # ---- g = a * (w1^T q) : fp8 [128, 2(kt=c-slice), NSH] ----
    g8 = pb.tile([128, 2, NSH], fp8, tag="g8", name="g8")
    for cs in range(2):
        gp = sp.tile([128, 1024], f32, tag="sp", name="gp")
        for h in range(2):
            nc.tensor.matmul(
                gp[:, h * 512:(h + 1) * 512],
                w1p8[:, cs],
                q2[:, :, h * 512:(h + 1) * 512],
                start=True, stop=True, perf_mode=DR,
            )
        for h in range(2):
            dst = g8[:, cs, h * 512:(h + 1) * 512]
            gsl = gp[:, h * 512:(h + 1) * 512]
            if (cs + h) % 2 == 0:
                nc.scalar.activation(dst, gsl, AF.Copy, scale=a_t[cs][:])
            else:
                nc.vector.tensor_scalar_mul(dst, gsl, a_t[cs][:])

    if PHASE == 2:
        _early_out([xq[:, 0], xq[:, 1]])
        ctx.close()
        return

    # ---- attention ----
    yts = [pb.tile([128, NSH], f32, tag=f"yts{t}", name=f"yts{t}")
           for t in range(2)]
    # attnx8[p, t(kt for proj), half, n] -- normalized attention-averaged x
    attnx8 = pb.tile([128, 2, 2, 512], fp8, tag="attnx8", name="attnx8")

    def emit_s_exp(half, pr):
        # S^T pair: S[m, n] = sum_c x[c, m] g[c, n], then exp -> E chunk
        st = sp.tile([128, 1024], f32, tag="sp", name="st")
        for h in range(2):
            nc.tensor.matmul(
                st[:, h * 512:(h + 1) * 512],
                xs8[:, 2 * pr + h],
                g8[:, :, half * 512:(half + 1) * 512],
                start=True, stop=True, perf_mode=DR,
            )
        # exp -> fp8e5 E chunk, permuted out to [p, ns, kt(2 mt), j]
        e = pw.tile([128, 4, 2, 128], u8, tag="e", name="e", bufs=5)
        stv = st[:].rearrange("p (kt ns j) -> p kt ns j", kt=2, ns=4)
        ev = e[:].rearrange("p ns kt j -> p kt ns j")
        a = EXP_SPLIT[half * 16 + pr]
        if a > 0:
            nc.scalar.activation(ev.bitcast(fp8e5)[:, :, 0:a, :],
                                 stv[:, :, 0:a, :], AF.Exp, scale=SCALE)
        if a < 4:
            nc.vector.tensor_scalar(ev[:, :, a:4, :], stv[:, :, a:4, :],
                                    SCALE * 4.0 * LOG2E, 60.0,
                                    op0=ALU.mult, op1=ALU.add)
        return e

    def emit_avx(half, pr, e, ots):
        # AVx accumulation + denominator columns (in ots[0] cols 256:260)
        xr = xTw8[:, pr].rearrange("p t kt j -> p kt t j")
        for ns in range(4):
            el = e[:, ns].bitcast(fp8e5)
            nc.tensor.matmul(
                ots[ns][:, 0:256], el, xr,
                start=(pr == 0), stop=(pr == 15), perf_mode=DR,
            )
            nc.tensor.matmul(
                ots[0][:, 256 + ns:257 + ns], el, ones5[:],
                start=(pr == 0), stop=(pr == 15), perf_mode=DR,
            )


    def finish_steps(half, ots):
        # generator of emission steps; each step is interleaved between the
        # next half's pairs so no engine queue sees a serial finish block.
        rec4 = ptiny.tile([128, 4], f32, tag="rec4", name="rec4")
        nc.vector.reciprocal(rec4[:], ots[0][:, 256:260])
        trps = [None, None]
        ons = []

        def emit_on(ns):
            on = pw.tile([128, 256], f32, tag="on", name="on", bufs=5)
            if ON_ACT[half * 4 + ns]:
                nc.scalar.activation(on[:], ots[ns][:, 0:256], AF.Identity,
                                     scale=rec4[:, ns:ns + 1])
            else:
                nc.vector.tensor_scalar_mul(on[:], ots[ns][:, 0:256],
                                            rec4[:, ns:ns + 1])
            ons.append(on)
            if ns == 0:
                trps[0] = ot.tile([128, 512], f32, tag="warm",
                                  name=f"trp{half}_0")
                trps[1] = ot.tile([128, 512], f32, tag="warm",
                                  name=f"trp{half}_1")
            for t in range(2):
                nc.tensor.transpose(trps[t][:, ns * 128:(ns + 1) * 128],
                                    ons[ns][:, t * 128:(t + 1) * 128], ident)

        def emit_drain(t):
            dst = attnx8[:, t, half]
            if TRP_ACT[half * 2 + t]:
                nc.scalar.activation(dst, trps[t][:], AF.Copy)
            else:
                nc.vector.tensor_copy(dst, trps[t][:])

        pjs = [None, None]

        def emit_preload(oh):
            # residual preload: exact fp32 identity matmul of xqb*WS into the
            # projection psum; the DR projection then accumulates on top.
            pj = ot.tile([128, 512], f32, tag="warm", name=f"pj{half}_{oh}")
            nc.tensor.matmul(pj[:], identw,
                             xqb[:, oh, half * 512:(half + 1) * 512],
                             start=True, stop=False, skip_group_check=True)
            pjs[oh] = pj

        def emit_proj(oh):
            # half 0: finish overlaps half 1's pairs -> keep the heavy ops on
            # PE (preload done) and split the 1-scalar drains ACT/DVE.
            pj = pjs[oh]
            nc.tensor.matmul(pj[:], w238[:, oh], attnx8[:, :, half],
                             start=False, stop=True, perf_mode=DR,
                             skip_group_check=True)
            sl = yts[oh][:, half * 512:(half + 1) * 512]
            if (half + oh) % 2 == 0:
                nc.scalar.activation(sl, pj[:], AF.Copy, scale=1.0 / WS)
            else:
                nc.vector.tensor_scalar_mul(sl, pj[:], 1.0 / WS)
            dq = (nc.sync, nc.scalar)[(half + oh) % 2]
            dq.dma_start(
                d["y"][:, oh * NSH + half * 512:oh * NSH + (half + 1) * 512],
                sl,
            )

        def emit_proj_stt(oh):
            # tail half: DVE is idle -> plain projection + stt keeps the
            # in-order PE stream short.
            pj = ot.tile([128, 512], f32, tag="warm", name=f"pj{half}_{oh}")
            nc.tensor.matmul(pj[:], w238[:, oh], attnx8[:, :, half],
                             start=True, stop=True, perf_mode=DR)
            sl = yts[oh][:, half * 512:(half + 1) * 512]
            nc.vector.scalar_tensor_tensor(
                sl, pj[:], 1.0 / WS, xqb[:, oh, half * 512:(half + 1) * 512],
                op0=ALU.mult, op1=ALU.add,
            )
            dq = (nc.sync, nc.scalar)[(half + oh) % 2]
            dq.dma_start(
                d["y"][:, oh * NSH + half * 512:oh * NSH + (half + 1) * 512],
                sl,
            )

        yield lambda: emit_on(0)
        yield lambda: emit_on(1)
        yield lambda: emit_on(2)
        yield lambda: emit_on(3)
        yield lambda: emit_drain(0)
        yield lambda: emit_drain(1)
        if half == 0:
            yield lambda: emit_preload(0)
            yield lambda: emit_preload(1)
            yield lambda: emit_proj(0)
            yield lambda: emit_proj(1)
        else:
            yield lambda: emit_proj_stt(0)
            yield lambda: emit_proj_stt(1)

    pending = None
    for half in range(2):
        ots = [ot.tile([128, 512], f32, tag="warm", name=f"ot{half}_{ns}")
               for ns in range(4)]
        e_cur = emit_s_exp(half, 0)
        for pr in range(16):
            e_nxt = emit_s_exp(half, pr + 1) if pr + 1 < 16 else None
            emit_avx(half, pr, e_cur, ots)
            e_cur = e_nxt
            if pending is not None:
                nxt = next(pending, None)
                if nxt is None:
                    pending = None
                else:
                    nxt()
        pending = finish_steps(half, ots)
    for step in pending:
        step()

    ctx.close()


_CACHE = {}


def _get_program():
    if "nc" in _CACHE:
        return _CACHE["nc"], _CACHE["dram"]
    nc = bacc.Bacc("TRN2", target_bir_lowering=False, debug=False,
                   enable_asserts=False, num_devices=NCORES)
    d = {}
    d["xs8"] = nc.dram_tensor("xs8", [128, MT * 256], fp8,
                              kind="ExternalInput").ap()
    d["xTw8"] = nc.dram_tensor("xTw8", [128, 16 * 512], fp8,
                               kind="ExternalInput").ap()
    d["xq8"] = nc.dram_tensor("xq8", [128, 2 * NSH], fp8,
                              kind="ExternalInput").ap()
    d["xq"] = nc.dram_tensor("xq", [128, 2 * NSH], f32, kind="ExternalInput").ap()
    d["wb"] = nc.dram_tensor("wb", [128, 4 * 128], bf16, kind="ExternalInput").ap()
    d["w1p8"] = nc.dram_tensor("w1p8", [128, 4 * 128], fp8,
                               kind="ExternalInput").ap()
    d["w23t"] = nc.dram_tensor("w23t", [128, 4 * 128], bf16,
                               kind="ExternalInput").ap()
    d["cpack"] = nc.dram_tensor("cpack", [128, CPW], f32,
                                kind="ExternalInput").ap()
    d["y"] = nc.dram_tensor("y", [128, 2 * NSH], f32, kind="ExternalOutput").ap()

    with tile.TileContext(nc) as tc:
        _build_body(nc, tc, d)
    nc.compile()
    _CACHE["nc"] = nc
    _CACHE["dram"] = d
    return nc, d


def make_in_maps(x, gamma, beta, w0, b0, w1, b1, w2, b2, w3, b3):
    """Host-side sharding/layout prep: returns list of 8 per-core inputs."""
    e4 = ml_dtypes.float8_e4m3
    xb = np.ascontiguousarray(np.asarray(x, np.float32).reshape(B, C, N))

    cpack = np.zeros((128, CPW), np.float32)
    gamma = np.asarray(gamma, np.float32)
    beta = np.asarray(beta, np.float32)
    b0 = np.asarray(b0, np.float32)
    bout = (np.asarray(w3, np.float32) @ np.asarray(b2, np.float32)
            + np.asarray(b3, np.float32))
    for t, off in ((0, CV0), (1, CV1)):
        sl = slice(t * 128, (t + 1) * 128)
        cpack[:, off + 0] = gamma[sl]
        cpack[:, off + 1] = beta[sl]
        cpack[:, off + 2] = b0[sl]
        cpack[:, off + 3] = bout[sl]
        cpack[:, off + 4] = -gamma[sl]
    for t, off in ((0, GMA), (1, GMB)):
        ch = t * 128 + np.arange(128)
        cpack[np.arange(128), off + ch // CPG] = 1.0 / NPG
    for t, off in ((0, GTA), (1, GTB)):
        ch = t * 128 + np.arange(128)
        cpack[ch // CPG, off + np.arange(128)] = 1.0
    cpack[:, IDT:IDT + 128] = np.eye(128, dtype=np.float32)
    cpack[:, IDW:IDW + 128] = np.eye(128, dtype=np.float32) * WS
    cpack[:, EPC] = EPS

    # wb[p, oh, kt, j] = w0^T[kt*128+p, oh*128+j]
    w0t = np.asarray(w0, np.float32).T.reshape(2, 128, 2, 128)  # [kt, p, oh, j]
    wb = w0t.transpose(1, 2, 0, 3).reshape(128, -1).astype(ml_dtypes.bfloat16)
    # w1p8[p, cs, kto, j] = w1[kto*128+p, cs*128+j]
    w1a = np.asarray(w1, np.float32).reshape(2, 128, 2, 128)    # [kto, p, cs, j]
    w1p8 = w1a.transpose(1, 2, 0, 3).reshape(128, -1).astype(e4)
    w23 = (np.asarray(w3, np.float32) @ np.asarray(w2, np.float32)).T
    w23t = w23.reshape(2, 128, 2, 128).transpose(1, 2, 0, 3)
    w23t = w23t.reshape(128, -1).astype(ml_dtypes.bfloat16)

    in_maps = []
    for core in range(NCORES):
        b, j = divmod(core, 4)
        xc = xb[b]
        xs8 = xc.reshape(2, 128, MT, 128).transpose(1, 2, 0, 3)
        xT = xc.reshape(2, 128, 16, 2, 128)  # [t, jj, pr, kt, p]
        xT = xT.transpose(4, 2, 0, 3, 1)
        xqc = xc[:, j * NSH:(j + 1) * NSH]
        xq = xqc.reshape(2, 128, NSH).transpose(1, 0, 2).reshape(128, -1)
        m = {
            "xs8": xs8.reshape(128, -1).astype(e4),
            "xTw8": xT.reshape(128, -1).astype(e4),
            "xq8": xq.astype(e4),
            "xq": np.ascontiguousarray(xq),
            "wb": wb, "w1p8": w1p8, "w23t": w23t, "cpack": cpack,
        }
        in_maps.append(m)
    return in_maps


def assemble_output(results):
    out = np.zeros((B, C, N), np.float32)
    for core in range(NCORES):
        b, j = divmod(core, 4)
        y = results[core]["y"].reshape(128, 2, NSH).transpose(1, 0, 2)
        out[b][:, j * NSH:(j + 1) * NSH] = y.reshape(C, NSH)
    return out.reshape(B, C, 16, 16, 16)


def kernel(x, gamma, beta, w0, b0, w1, b1, w2, b2, w3, b3):
    nc, _ = _get_program()
    in_maps = make_in_maps(x, gamma, beta, w0, b0, w1, b1, w2, b2, w3, b3)
    res = bass_utils.run_bass_kernel_spmd(nc, in_maps, core_ids=list(range(NCORES)))
    return assemble_output(res.results)


# revision 23
# speedup vs baseline: 1.0777x; 1.0777x over previous
"""Trainium2 Bass/Tile kernel for AttnBlock:
GroupNorm(32) -> 1x1 conv q,k,v -> softmax attention over N=4096 tokens
-> 1x1 conv proj -> residual.

Sharding: 8 cores = 2 (batch) x 4 (query-token shards of N).  Each core gets
the full x of its batch plus its n-shard slice, and produces the [C, N/4]
output shard.  No collectives.

Architecture (v4):
- All heavy matmuls are fp8 MatmulPerfMode.DoubleRow: the full K=256
  contraction in one instruction at 0.5 cycles/output-column.  DR stationary
  operands need their 256 weight elements contiguous per partition; every
  lhsT is laid out [.., kt(2), 128].
- GroupNorm stats via a PE Gram-matrix over the m-major fp8 x copy
  (diag -> sum x^2, ones-matmul -> sum x), diag extracted by one DVE
  scalar_tensor_tensor+accum per c-tile.
- No k tensor: S^T = x^T g with g = a*(w1^T q) [C, NSH] -- the PSUM->SBUF
  drain is the n-shard-sized g (2K lanes) instead of the m-sized k (8K).
  The k bias is dropped exactly (softmax shift invariance); q keeps its
  effective bias.
- No v tensor: attention accumulates over x itself:
  AVx[n, c] = sum_m E[m, n] x[c, m] (moving operand = resident xTw8),
  plus denominator columns from a tiny ones matmul per ns.  After
  normalize + transpose, ONE DoubleRow projection by w238 = a*(w3 w2)^T
  (host-folded w3@w2, scaled 2^19 for fp8) produces the output; the scale
  is undone in the final scalar_tensor_tensor against xqb = x + b3eff.
- Softmax over 2-bank [128,1024] S^T psum tiles; exp ns-subtiles split
  between ACT (true Exp -> fp8e5) and DVE (Schraudolph bits =
  round(logit*4*log2e + 60) as uint8 == fp8e5m2; e5m2 because logits span
  +-8).  Output APs are permuted so E tiles come out [ns, kt, j] -- the
  DR lhsT layout for AVx.
"""

import ml_dtypes
import numpy as np

import concourse.bacc as bacc
import concourse.mybir as mybir
import concourse.tile as tile
from concourse import bass_utils

f32 = mybir.dt.float32
bf16 = mybir.dt.bfloat16
fp8 = mybir.dt.float8e4
fp8e5 = mybir.dt.float8e5
u8 = mybir.dt.uint8
AF = mybir.ActivationFunctionType
ALU = mybir.AluOpType
DR = mybir.MatmulPerfMode.DoubleRow

B = 2
C = 256
N = 4096          # 16**3 tokens
NSH = N // 4      # 1024 tokens per core
G = 32
CPG = C // G      # channels per group
NPG = CPG * N     # elements per group
EPS = 1e-6
SCALE = C ** -0.5          # 1/16
LOG2E = float(1.0 / np.log(2.0))
WS = 524288.0              # 2^19 fp8-range scale on w23; undone in the stt
MT = N // 128              # 32 m-tiles

NCORES = 8

# cpack column layout
CV0, CV1 = 0, 8            # cvec slice0/1: [gamma, beta, b0, bout, -gamma]
GMA, GMB = 16, 48          # gmask per slice [128, 32] (1/NPG folded)
GTA, GTB = 80, 208         # gmaskT per slice [32, 128] on partitions 0:32
MZL = 336                  # zeros[128] | ident[128] | zeros[128]
IDT = 464
EPC = 720                  # eps column
IDW = 728                  # identity * WS (residual preload)
CPW = 856

# engine splits (True -> ACT, False -> DVE)
EXP_SPLIT = [2] * 32       # of 4 ns-subtiles per (half*16+pair), how many ACT
GEP_ACT = [True, False]    # g drain per c-slice
ON_ACT = [True, False, True, False] * 2  # normalize per (half*4 + ns)
TRP_ACT = [True, False, True, False]  # attnx drain per (half*2 + t)

N_WARMUP = 42
PHASE = 4


def _build_body(nc, tc, d):
    from contextlib import ExitStack

    ctx = ExitStack()
    pc = ctx.enter_context(tc.tile_pool(name="const", bufs=1))
    pb = ctx.enter_context(tc.tile_pool(name="big", bufs=1))
    pw = ctx.enter_context(tc.tile_pool(name="work", bufs=3))
    ptiny = ctx.enter_context(tc.tile_pool(name="tiny", bufs=2))
    # PSUM: sp = 2 x [128,1024] (2 banks each), ot = 4 x [128,512] (1 bank)
    sp = ctx.enter_context(tc.tile_pool(name="sp", bufs=2, space="PSUM"))
    ot = ctx.enter_context(tc.tile_pool(name="pot", bufs=4, space="PSUM"))

    # ---- tiny consts ----
    zcol = pc.tile([128, 1], f32, tag="zcol", name="zcol")
    nc.vector.memset(zcol[:], 0.0)
    nc.const_aps.aps[(f32, 0.0)] = zcol[:]
    ones4 = pc.tile([128, 2, 1], fp8, tag="ones4", name="ones4")
    nc.vector.memset(ones4[:], 1.0)
    ones5 = pc.tile([128, 2, 1], fp8e5, tag="ones5", name="ones5")
    nc.vector.memset(ones5[:], 1.0)

    # ---- PE warmup: dep-free matmuls bridge the DMA head + pstate ramp
    wdum = pc.tile([128, 128], bf16, tag="wdum", name="wdum")
    nc.vector.memset(wdum[:], 1.0)
    wslot = ot.tile([128, 512], f32, tag="warm", name="warm")
    for i in range(N_WARMUP):
        nc.tensor.matmul(wslot[:, 0:128], wdum[:], wdum[:],
                         start=True, stop=True)

    # ---- input DMAs: the DMA fabric is serial -- order by need.
    # xTw8[p, pr, t, kt, j] = x[t*128+j, (2*pr+kt)*128+p], in quarters
    xTw8 = pb.tile([128, 16, 2, 2, 128], fp8, tag="xTw8", name="xTw8")
    xTw8f = xTw8[:].rearrange("p a b c e -> p (a b c e)")
    qs = [nc.sync, nc.scalar]
    cpack = pc.tile([128, CPW], f32, tag="cpack", name="cpack")
    for qr in range(4):
        qs[qr % 2].dma_start(xTw8f[:, qr * 2048:(qr + 1) * 2048],
                             d["xTw8"][:, qr * 2048:(qr + 1) * 2048])
        if qr == 1:
            nc.sync.dma_start(cpack[:], d["cpack"][:])
        if qr == 2:
            xq8 = pb.tile([128, 2, NSH], fp8, tag="xq8", name="xq8")
            nc.scalar.dma_start(xq8[:].rearrange("p a b -> p (a b)"),
                                d["xq8"][:])
    # wb[p, oh, kt, j] = w0^T[kt*128+p, oh*128+j]
    wb = pb.tile([128, 2, 2, 128], bf16, tag="wb", name="wb")
    nc.sync.dma_start(wb[:].rearrange("p a b c -> p (a b c)"), d["wb"][:])
    # w1p8[p, cs, kto, j] = w1[kto*128+p, cs*128+j]  (plain w1, fp8)
    w1p8 = pb.tile([128, 2, 2, 128], fp8, tag="w1p8", name="w1p8")
    nc.scalar.dma_start(w1p8[:].rearrange("p a b c -> p (a b c)"), d["w1p8"][:])
    # w23t[p, oh, kt, j] = (w3 w2)^T[kt*128+p, oh*128+j]
    w23t = pb.tile([128, 2, 2, 128], bf16, tag="w23t", name="w23t")
    nc.sync.dma_start(w23t[:].rearrange("p a b c -> p (a b c)"), d["w23t"][:])
    # xs8[p, mt, kt, j] = x[kt*128+p, mt*128+j], halves
    xs8 = pb.tile([128, MT, 2, 128], fp8, tag="xs8", name="xs8")
    xs8f = xs8[:].rearrange("p a b c -> p (a b c)")
    nc.scalar.dma_start(xs8f[:, 0:4096], d["xs8"][:, 0:4096])
    nc.sync.dma_start(xs8f[:, 4096:8192], d["xs8"][:, 4096:8192])
    # xq (f32 residual) is emitted LAST -- only needed by the final stt

    cvec = [cpack[:, CV0:CV0 + 8], cpack[:, CV1:CV1 + 8]]
    gm = [cpack[:, GMA:GMA + 32], cpack[:, GMB:GMB + 32]]
    gmt = [cpack[0:32, GTA:GTA + 128], cpack[0:32, GTB:GTB + 128]]
    ident = cpack[:, IDT:IDT + 128]
    dmask = [cpack[:, IDT:IDT + 256], cpack[:, MZL:MZL + 256]]
    identw = cpack[:, IDW:IDW + 128]
    epscol = cpack[0:32, EPC:EPC + 1]

    # ---- GroupNorm stats via PE Gram over xTw8 ----
    # gtile: [t0 gram 0:256 | t0 sum-x 256 | pad | t1 gram 512:768 | t1 sum-x]
    gtile = sp.tile([128, 1024], f32, tag="sp", name="gram")
    goff = [0, 512]
    for pr in range(16):
        for t in range(2):
            lhs = xTw8[:, pr, t]
            nc.tensor.matmul(
                gtile[:, goff[t]:goff[t] + 256],
                lhs,
                xTw8[:, pr].rearrange("p t kt j -> p kt t j"),
                start=(pr == 0), stop=(pr == 15), perf_mode=DR,
            )
            nc.tensor.matmul(
                gtile[:, goff[t] + 256:goff[t] + 257],
                lhs, ones4[:],
                start=(pr == 0), stop=(pr == 15), perf_mode=DR,
            )
    # pt[t]: col0 = sum x, col1 = sum x^2 (diag extract)
    pt = [ptiny.tile([128, 2], f32, tag=f"pt{t}", name=f"pt{t}") for t in range(2)]
    trash = pw.tile([128, 256], f32, tag="trash", name="trash", bufs=2)
    for t in range(2):
        nc.vector.tensor_copy(pt[t][:, 0:1], gtile[:, goff[t] + 256:goff[t] + 257])
        nc.vector.scalar_tensor_tensor(
            trash[:], gtile[:, goff[t]:goff[t] + 256], 1.0, dmask[t],
            op0=ALU.mult, op1=ALU.mult, accum_out=pt[t][:, 1:2],
        )

    # group stats -> mu/rstd -> per-channel a, bfold
    s32 = ot.tile([128, 512], f32, tag="warm", name="s32")
    for t in range(2):
        nc.tensor.matmul(s32[0:32, 0:2], gm[t], pt[t][:],
                         start=(t == 0), stop=(t == 1))
    sg = ptiny.tile([32, 2], f32, tag="sg", name="sg")
    nc.vector.tensor_copy(sg[:], s32[0:32, 0:2])
    mr = ptiny.tile([32, 2], f32, tag="mr", name="mr")
    musq = ptiny.tile([32, 1], f32, tag="musq", name="musq")
    nc.vector.tensor_mul(musq[:], sg[:, 0:1], sg[:, 0:1])
    var = ptiny.tile([32, 1], f32, tag="var", name="var")
    nc.vector.tensor_sub(var[:], sg[:, 1:2], musq[:])
    std = ptiny.tile([32, 1], f32, tag="std", name="std")
    nc.scalar.activation(std[:], var[:], AF.Sqrt, bias=epscol)
    # dummy Exp preloads the exp table during the DMA head
    edum = ptiny.tile([128, 1], u8, tag="edum", name="edum")
    nc.scalar.activation(edum[:].bitcast(fp8e5), zcol[:], AF.Exp)
    nc.vector.reciprocal(mr[:, 1:2], std[:])
    nc.vector.tensor_mul(mr[:, 0:1], sg[:, 0:1], mr[:, 1:2])

    a_t, bfold_bf = [], []
    bc = ot.tile([128, 512], f32, tag="warm", name="bc")
    for t in range(2):
        nc.tensor.matmul(bc[:, 2 * t:2 * t + 2], gmt[t], mr[:],
                         start=True, stop=True)
    for t in range(2):
        a = pb.tile([128, 1], f32, tag=f"a{t}", name=f"a{t}")
        nc.vector.tensor_mul(a[:], bc[:, 2 * t + 1:2 * t + 2], cvec[t][:, 0:1])
        bf = pb.tile([128, 1], f32, tag=f"bf{t}", name=f"bf{t}")
        nc.vector.tensor_scalar(
            bf[:], bc[:, 2 * t:2 * t + 1], cvec[t][:, 4:5], cvec[t][:, 1:2],
            op0=ALU.mult, op1=ALU.add,
        )
        bb = pb.tile([128, 1], bf16, tag=f"bfb{t}", name=f"bfb{t}")
        nc.vector.tensor_copy(bb[:], bf[:])
        a_t.append(a)
        bfold_bf.append(bb)

    def _early_out(srcs):
        for oh in range(2):
            for ch in range(2):
                yt = pw.tile([128, 512], f32, tag="yt", name="yt")
                nc.vector.tensor_copy(
                    yt[:], srcs[oh][:, ch * 512:(ch + 1) * 512])
                nc.sync.dma_start(
                    d["y"][:, oh * NSH + ch * 512:oh * NSH + (ch + 1) * 512],
                    yt[:])

    # ---- effective biases (RAW weights -- emitted before the a-fold) ----
    beff0 = []
    for oh in range(2):
        bp = ot.tile([128, 512], f32, tag="warm", name="bp")
        for t in range(2):
            nc.tensor.matmul(bp[:, 0:1], wb[:, oh, t],
                             bfold_bf[t][:], start=(t == 0), stop=(t == 1))
        bs = pb.tile([128, 1], f32, tag=f"beff0_{oh}", name=f"beff0_{oh}")
        nc.scalar.activation(bs[:], bp[:, 0:1], AF.Identity,
                             bias=cvec[oh][:, 2:3])
        beff0.append(bs)
    b3eff = []
    for oh in range(2):
        bp = ot.tile([128, 512], f32, tag="warm", name="bp3")
        for t in range(2):
            nc.tensor.matmul(bp[:, 0:1], w23t[:, oh, t],
                             bfold_bf[t][:], start=(t == 0), stop=(t == 1))
        bs = pb.tile([128, 1], f32, tag=f"b3eff{oh}", name=f"b3eff{oh}")
        nc.scalar.activation(bs[:], bp[:, 0:1], AF.Identity,
                             bias=cvec[oh][:, 3:4])
        b3eff.append(bs)

    # xqb = x-shard + b3eff (f32); the final stt adds proj*2^-19 onto it.
    # xq is the last input DMA issued -- its transfer queues behind all the
    # early-needed inputs on the serial DMA fabric but lands well before the
    # first finish phase needs xqb.
    xq = pb.tile([128, 2, NSH], f32, tag="xq", name="xq")
    nc.sync.dma_start(xq[:].rearrange("p a b -> p (a b)"), d["xq"][:])
    xqb = pb.tile([128, 2, NSH], f32, tag="xqb", name="xqb")

    def emit_xqb(t):
        # on gpsimd: its queue is idle, so blocking on the late xq DMA is
        # free (on DVE the scheduler head-of-line-blocked the queue)
        nc.gpsimd.tensor_scalar(xqb[:, t], xq[:, t], b3eff[t][:], None,
                                op0=ALU.add)

    # ---- fold a into the fp8 weights directly (one DVE op each) ----
    w018 = pb.tile([128, 2, 2, 128], fp8, tag="w018", name="w018")
    w238 = pb.tile([128, 2, 2, 128], fp8, tag="w238", name="w238")
    for t in range(2):
        nc.vector.tensor_scalar_mul(w018[:, :, t], wb[:, :, t], a_t[t][:])
        nc.vector.tensor_scalar(w238[:, :, t], w23t[:, :, t], a_t[t][:], WS,
                                op0=ALU.mult, op1=ALU.mult)

    if PHASE <= 1:
        _early_out([xq[:, 0], xq[:, 1]])
        ctx.close()
        return

    # ---- q = w0a @ xq + beff0 : fp8 [128, 2(kt=oh), NSH] ----
    # q and g run through the 4-slot ot pool so the 2 big sp slots stay free
    # for the attention pipeline (the sp ring was serializing q -> g -> S).
    q2 = pb.tile([128, 2, NSH], fp8, tag="q2", name="q2")
    for oh in range(2):
        for ch in range(2):
            qp = ot.tile([128, 512], f32, tag="warm", name=f"qp{oh}{ch}")
            nc.tensor.matmul(
                qp[:],
                w018[:, oh],
                xq8[:, :, ch * 512:(ch + 1) * 512],
                start=True, stop=True, perf_mode=DR,
            )
            dst = q2[:, oh, ch * 512:(ch + 1) * 512]
            if (oh + ch) % 2 == 0:
                nc.scalar.activation(dst, qp[:], AF.Identity,
                                     bias=beff0[oh][:])
            else:
                nc.vector.tensor_scalar(dst, qp[:], beff0[oh][:], None,
                                        op0=ALU.add)

    # ---- g = a * (w1^T q) : fp8 [128, 2(kt=c-slice), NSH] ----
    g8 = pb.tile([128, 2, NSH], fp8, tag="g8", name="g8")
    for h in range(2):
        for cs in range(2):
            gp = ot.tile([128, 512], f32, tag="warm", name=f"gp{cs}{h}")
            nc.tensor.matmul(
                gp[:],
                w1p8[:, cs],
                q2[:, :, h * 512:(h + 1) * 512],
                start=True, stop=True, perf_mode=DR,
            )
            dst = g8[:, cs, h * 512:(h + 1) * 512]
            if (cs + h) % 2 == 0:
                nc.scalar.activation(dst, gp[:], AF.Copy, scale=a_t[cs][:])
            else:
                nc.vector.tensor_scalar_mul(dst, gp[:], a_t[cs][:])

    if PHASE == 2:
        _early_out([xq[:, 0], xq[:, 1]])
        ctx.close()
        return

    # ---- attention ----
    yts = [pb.tile([128, NSH], f32, tag=f"yts{t}", name=f"yts{t}")
           for t in range(2)]
    # attnx8[p, t(kt for proj), half, n] -- normalized attention-averaged x
    attnx8 = pb.tile([128, 2, 2, 512], fp8, tag="attnx8", name="attnx8")

    def emit_s_exp(half, pr):
        # S^T pair: S[m, n] = sum_c x[c, m] g[c, n], then exp -> E chunk
        st = sp.tile([128, 1024], f32, tag="sp", name="st")
        for h in range(2):
            nc.tensor.matmul(
                st[:, h * 512:(h + 1) * 512],
                xs8[:, 2 * pr + h],
                g8[:, :, half * 512:(half + 1) * 512],
                start=True, stop=True, perf_mode=DR,
            )
        # exp -> fp8e5 E chunks, permuted out to [p, ns, kt(2 mt), j].
        # Separate tiles per engine: co-writing one tile through permuted
        # views serializes ACT->DVE in the scheduler.
        stv = st[:].rearrange("p (kt ns j) -> p kt ns j", kt=2, ns=4)
        e1 = pw.tile([128, 2, 2, 128], u8, tag="e1", name="e1", bufs=5)
        e2 = pw.tile([128, 2, 2, 128], u8, tag="e2", name="e2", bufs=5)
        ev1 = e1[:].rearrange("p ns kt j -> p kt ns j")
        ev2 = e2[:].rearrange("p ns kt j -> p kt ns j")
        if EXP_SPLIT[half * 16 + pr] >= 2:
            nc.scalar.activation(ev1.bitcast(fp8e5), stv[:, :, 0:2, :],
                                 AF.Exp, scale=SCALE)
            nc.vector.tensor_scalar(ev2, stv[:, :, 2:4, :],
                                    SCALE * 4.0 * LOG2E, 60.0,
                                    op0=ALU.mult, op1=ALU.add)
        else:
            nc.vector.tensor_scalar(ev1, stv[:, :, 0:2, :],
                                    SCALE * 4.0 * LOG2E, 60.0,
                                    op0=ALU.mult, op1=ALU.add)
            nc.scalar.activation(ev2.bitcast(fp8e5), stv[:, :, 2:4, :],
                                 AF.Exp, scale=SCALE)
        return (e1, e2)

    def emit_avx(half, pr, e, ots):
        # AVx accumulation + denominator columns (in ots[0] cols 256:260)
        e1, e2 = e
        xr = xTw8[:, pr].rearrange("p t kt j -> p kt t j")
        for ns in range(4):
            et = e1 if ns < 2 else e2
            el = et[:, ns % 2].bitcast(fp8e5)
            nc.tensor.matmul(
                ots[ns][:, 0:256], el, xr,
                start=(pr == 0), stop=(pr == 15), perf_mode=DR,
            )
            nc.tensor.matmul(
                ots[0][:, 256 + ns:257 + ns], el, ones5[:],
                start=(pr == 0), stop=(pr == 15), perf_mode=DR,
            )


    def finish_steps(half, ots):
        # generator of emission steps; each step is interleaved between the
        # next half's pairs so no engine queue sees a serial finish block.
        rec4 = ptiny.tile([128, 4], f32, tag="rec4", name="rec4")
        nc.vector.reciprocal(rec4[:], ots[0][:, 256:260])
        trps = [None, None]
        ons = []

        def emit_on(ns):
            on = pw.tile([128, 256], f32, tag="on", name="on", bufs=5)
            if ON_ACT[half * 4 + ns]:
                nc.scalar.activation(on[:], ots[ns][:, 0:256], AF.Identity,
                                     scale=rec4[:, ns:ns + 1])
            else:
                nc.vector.tensor_scalar_mul(on[:], ots[ns][:, 0:256],
                                            rec4[:, ns:ns + 1])
            ons.append(on)
            if ns == 0:
                trps[0] = ot.tile([128, 512], f32, tag="warm",
                                  name=f"trp{half}_0")
                trps[1] = ot.tile([128, 512], f32, tag="warm",
                                  name=f"trp{half}_1")
            for t in range(2):
                nc.tensor.transpose(trps[t][:, ns * 128:(ns + 1) * 128],
                                    ons[ns][:, t * 128:(t + 1) * 128], ident)

        def emit_drain(t):
            dst = attnx8[:, t, half]
            if TRP_ACT[half * 2 + t]:
                nc.scalar.activation(dst, trps[t][:], AF.Copy)
            else:
                nc.vector.tensor_copy(dst, trps[t][:])

        pjs = [None, None]

        def emit_preload(oh):
            # residual preload: exact fp32 identity matmul of xqb*WS into the
            # projection psum; the DR projection then accumulates on top.
            pj = ot.tile([128, 512], f32, tag="warm", name=f"pj{half}_{oh}")
            nc.tensor.matmul(pj[:], identw,
                             xqb[:, oh, half * 512:(half + 1) * 512],
                             start=True, stop=False, skip_group_check=True)
            pjs[oh] = pj

        def emit_proj(oh):
            # half 0: finish overlaps half 1's pairs -> keep the heavy ops on
            # PE (preload done) and split the 1-scalar drains ACT/DVE.
            pj = pjs[oh]
            nc.tensor.matmul(pj[:], w238[:, oh], attnx8[:, :, half],
                             start=False, stop=True, perf_mode=DR,
                             skip_group_check=True)
            sl = yts[oh][:, half * 512:(half + 1) * 512]
            if (half + oh) % 2 == 0:
                nc.scalar.activation(sl, pj[:], AF.Copy, scale=1.0 / WS)
            else:
                nc.vector.tensor_scalar_mul(sl, pj[:], 1.0 / WS)
            dq = (nc.sync, nc.scalar)[(half + oh) % 2]
            dq.dma_start(
                d["y"][:, oh * NSH + half * 512:oh * NSH + (half + 1) * 512],
                sl,
            )

        def emit_proj_stt(oh):
            # tail half: DVE is idle -> plain projection + stt keeps the
            # in-order PE stream short.
            pj = ot.tile([128, 512], f32, tag="warm", name=f"pj{half}_{oh}")
            nc.tensor.matmul(pj[:], w238[:, oh], attnx8[:, :, half],
                             start=True, stop=True, perf_mode=DR)
            sl = yts[oh][:, half * 512:(half + 1) * 512]
            nc.vector.scalar_tensor_tensor(
                sl, pj[:], 1.0 / WS, xqb[:, oh, half * 512:(half + 1) * 512],
                op0=ALU.mult, op1=ALU.add,
            )
            dq = (nc.sync, nc.scalar)[(half + oh) % 2]
            dq.dma_start(
                d["y"][:, oh * NSH + half * 512:oh * NSH + (half + 1) * 512],
                sl,
            )

        yield lambda: emit_on(0)
        yield lambda: emit_on(1)
        yield lambda: emit_on(2)
        yield lambda: emit_on(3)
        yield lambda: emit_drain(0)
        yield lambda: emit_drain(1)
        if half == 0:
            yield lambda: emit_preload(0)
            yield lambda: emit_preload(1)
            yield lambda: emit_proj(0)
            yield lambda: emit_proj(1)
        else:
            yield lambda: emit_proj_stt(0)
            yield lambda: emit_proj_stt(1)

    pending = iter([lambda: emit_xqb(0), lambda: emit_xqb(1)])
    for half in range(2):
        ots = [ot.tile([128, 512], f32, tag="warm", name=f"ot{half}_{ns}")
               for ns in range(4)]
        e_cur = emit_s_exp(half, 0)
        for pr in range(16):
            e_nxt = emit_s_exp(half, pr + 1) if pr + 1 < 16 else None
            emit_avx(half, pr, e_cur, ots)
            e_cur = e_nxt
            if pending is not None:
                nxt = next(pending, None)
                if nxt is None:
                    pending = None
                else:
                    nxt()
        pending = finish_steps(half, ots)
    for step in pending:
        step()

    ctx.close()


_CACHE = {}


def _get_program():
    if "nc" in _CACHE:
        return _CACHE["nc"], _CACHE["dram"]
    nc = bacc.Bacc("TRN2", target_bir_lowering=False, debug=False,
                   enable_asserts=False, num_devices=NCORES)
    d = {}
    d["xs8"] = nc.dram_tensor("xs8", [128, MT * 256], fp8,
                              kind="ExternalInput").ap()
    d["xTw8"] = nc.dram_tensor("xTw8", [128, 16 * 512], fp8,
                               kind="ExternalInput").ap()
    d["xq8"] = nc.dram_tensor("xq8", [128, 2 * NSH], fp8,
                              kind="ExternalInput").ap()
    d["xq"] = nc.dram_tensor("xq", [128, 2 * NSH], f32, kind="ExternalInput").ap()
    d["wb"] = nc.dram_tensor("wb", [128, 4 * 128], bf16, kind="ExternalInput").ap()
    d["w1p8"] = nc.dram_tensor("w1p8", [128, 4 * 128], fp8,
                               kind="ExternalInput").ap()
    d["w23t"] = nc.dram_tensor("w23t", [128, 4 * 128], bf16,
                               kind="ExternalInput").ap()
    d["cpack"] = nc.dram_tensor("cpack", [128, CPW], f32,
                                kind="ExternalInput").ap()
    d["y"] = nc.dram_tensor("y", [128, 2 * NSH], f32, kind="ExternalOutput").ap()

    with tile.TileContext(nc) as tc:
        _build_body(nc, tc, d)
    nc.compile()
    _CACHE["nc"] = nc
    _CACHE["dram"] = d
    return nc, d


def make_in_maps(x, gamma, beta, w0, b0, w1, b1, w2, b2, w3, b3):
    """Host-side sharding/layout prep: returns list of 8 per-core inputs."""
    e4 = ml_dtypes.float8_e4m3
    xb = np.ascontiguousarray(np.asarray(x, np.float32).reshape(B, C, N))

    cpack = np.zeros((128, CPW), np.float32)
    gamma = np.asarray(gamma, np.float32)
    beta = np.asarray(beta, np.float32)
    b0 = np.asarray(b0, np.float32)
    bout = (np.asarray(w3, np.float32) @ np.asarray(b2, np.float32)
            + np.asarray(b3, np.float32))
    for t, off in ((0, CV0), (1, CV1)):
        sl = slice(t * 128, (t + 1) * 128)
        cpack[:, off + 0] = gamma[sl]
        cpack[:, off + 1] = beta[sl]
        cpack[:, off + 2] = b0[sl]
        cpack[:, off + 3] = bout[sl]
        cpack[:, off + 4] = -gamma[sl]
    for t, off in ((0, GMA), (1, GMB)):
        ch = t * 128 + np.arange(128)
        cpack[np.arange(128), off + ch // CPG] = 1.0 / NPG
    for t, off in ((0, GTA), (1, GTB)):
        ch = t * 128 + np.arange(128)
        cpack[ch // CPG, off + np.arange(128)] = 1.0
    cpack[:, IDT:IDT + 128] = np.eye(128, dtype=np.float32)
    cpack[:, IDW:IDW + 128] = np.eye(128, dtype=np.float32) * WS
    cpack[:, EPC] = EPS

    # wb[p, oh, kt, j] = w0^T[kt*128+p, oh*128+j]
    w0t = np.asarray(w0, np.float32).T.reshape(2, 128, 2, 128)  # [kt, p, oh, j]
    wb = w0t.transpose(1, 2, 0, 3).reshape(128, -1).astype(ml_dtypes.bfloat16)
    # w1p8[p, cs, kto, j] = w1[kto*128+p, cs*128+j]
    w1a = np.asarray(w1, np.float32).reshape(2, 128, 2, 128)    # [kto, p, cs, j]
    w1p8 = w1a.transpose(1, 2, 0, 3).reshape(128, -1).astype(e4)
    w23 = (np.asarray(w3, np.float32) @ np.asarray(w2, np.float32)).T
    w23t = w23.reshape(2, 128, 2, 128).transpose(1, 2, 0, 3)
    w23t = w23t.reshape(128, -1).astype(ml_dtypes.bfloat16)

    in_maps = []
    for core in range(NCORES):
        b, j = divmod(core, 4)
        xc = xb[b]
        xs8 = xc.reshape(2, 128, MT, 128).transpose(1, 2, 0, 3)
        xT = xc.reshape(2, 128, 16, 2, 128)  # [t, jj, pr, kt, p]
        xT = xT.transpose(4, 2, 0, 3, 1)
        xqc = xc[:, j * NSH:(j + 1) * NSH]
        xq = xqc.reshape(2, 128, NSH).transpose(1, 0, 2).reshape(128, -1)
        m = {
            "xs8": xs8.reshape(128, -1).astype(e4),
            "xTw8": xT.reshape(128, -1).astype(e4),
            "xq8": xq.astype(e4),
            "xq": np.ascontiguousarray(xq),
            "wb": wb, "w1p8": w1p8, "w23t": w23t, "cpack": cpack,
        }
        in_maps.append(m)
    return in_maps


def assemble_output(results):
    out = np.zeros((B, C, N), np.float32)
    for core in range(NCORES):
        b, j = divmod(core, 4)
        y = results[core]["y"].reshape(128, 2, NSH).transpose(1, 0, 2)
        out[b][:, j * NSH:(j + 1) * NSH] = y.reshape(C, NSH)
    return out.reshape(B, C, 16, 16, 16)


def kernel(x, gamma, beta, w0, b0, w1, b1, w2, b2, w3, b3):
    nc, _ = _get_program()
    in_maps = make_in_maps(x, gamma, beta, w0, b0, w1, b1, w2, b2, w3, b3)
    res = bass_utils.run_bass_kernel_spmd(nc, in_maps, core_ids=list(range(NCORES)))
    return assemble_output(res.results)


# revision 28
# speedup vs baseline: 1.0853x; 1.0071x over previous
"""Trainium2 Bass/Tile kernel for AttnBlock:
GroupNorm(32) -> 1x1 conv q,k,v -> softmax attention over N=4096 tokens
-> 1x1 conv proj -> residual.

Sharding: 8 cores = 2 (batch) x 4 (query-token shards of N).  Each core gets
the full x of its batch plus its n-shard slice, and produces the [C, N/4]
output shard.  No collectives.

Architecture (v4):
- All heavy matmuls are fp8 MatmulPerfMode.DoubleRow: the full K=256
  contraction in one instruction at 0.5 cycles/output-column.  DR stationary
  operands need their 256 weight elements contiguous per partition; every
  lhsT is laid out [.., kt(2), 128].
- GroupNorm stats via a PE Gram-matrix over the m-major fp8 x copy
  (diag -> sum x^2, ones-matmul -> sum x), diag extracted by one DVE
  scalar_tensor_tensor+accum per c-tile.
- No k tensor: S^T = x^T g with g = a*(w1^T q) [C, NSH] -- the PSUM->SBUF
  drain is the n-shard-sized g (2K lanes) instead of the m-sized k (8K).
  The k bias is dropped exactly (softmax shift invariance); q keeps its
  effective bias.
- No v tensor: attention accumulates over x itself:
  AVx[n, c] = sum_m E[m, n] x[c, m] (moving operand = resident xTw8),
  plus denominator columns from a tiny ones matmul per ns.  After
  normalize + transpose, ONE DoubleRow projection by w238 = a*(w3 w2)^T
  (host-folded w3@w2, scaled 2^19 for fp8) produces the output; the scale
  is undone in the final scalar_tensor_tensor against xqb = x + b3eff.
- Softmax over 2-bank [128,1024] S^T psum tiles; exp ns-subtiles split
  between ACT (true Exp -> fp8e5) and DVE (Schraudolph bits =
  round(logit*4*log2e + 60) as uint8 == fp8e5m2; e5m2 because logits span
  +-8).  Output APs are permuted so E tiles come out [ns, kt, j] -- the
  DR lhsT layout for AVx.
"""

import ml_dtypes
import numpy as np

import concourse.bacc as bacc
import concourse.mybir as mybir
import concourse.tile as tile
from concourse import bass_utils

f32 = mybir.dt.float32
bf16 = mybir.dt.bfloat16
fp8 = mybir.dt.float8e4
fp8e5 = mybir.dt.float8e5
u8 = mybir.dt.uint8
AF = mybir.ActivationFunctionType
ALU = mybir.AluOpType
DR = mybir.MatmulPerfMode.DoubleRow

B = 2
C = 256
N = 4096          # 16**3 tokens
NSH = N // 4      # 1024 tokens per core
G = 32
CPG = C // G      # channels per group
NPG = CPG * N     # elements per group
EPS = 1e-6
SCALE = C ** -0.5          # 1/16
LOG2E = float(1.0 / np.log(2.0))
WS = 524288.0              # 2^19 fp8-range scale on w23; undone in the stt
MT = N // 128              # 32 m-tiles

NCORES = 8

# cpack column layout
CV0, CV1 = 0, 8            # cvec slice0/1: [gamma, beta, b0, bout, -gamma]
GMA, GMB = 16, 48          # gmask per slice [128, 32] (1/NPG folded)
GTA, GTB = 80, 208         # gmaskT per slice [32, 128] on partitions 0:32
MZL = 336                  # zeros[128] | ident[128] | zeros[128]
IDT = 464
EPC = 720                  # eps column
IDW = 728                  # identity * WS (residual preload)
CPW = 856

# engine splits (True -> ACT, False -> DVE)
EXP_SPLIT = [2] * 32       # of 4 ns-subtiles per (half*16+pair), how many ACT
GEP_ACT = [True, False]    # g drain per c-slice
ON_ACT = [True, False, True, False] * 2  # normalize per (half*4 + ns)
TRP_ACT = [True, False, True, False]  # attnx drain per (half*2 + t)

N_WARMUP = 42
PHASE = 4


def _build_body(nc, tc, d):
    from contextlib import ExitStack

    ctx = ExitStack()
    pc = ctx.enter_context(tc.tile_pool(name="const", bufs=1))
    pb = ctx.enter_context(tc.tile_pool(name="big", bufs=1))
    pw = ctx.enter_context(tc.tile_pool(name="work", bufs=3))
    ptiny = ctx.enter_context(tc.tile_pool(name="tiny", bufs=2))
    # PSUM: sp = 2 x [128,1024] (2 banks each), ot = 4 x [128,512] (1 bank)
    sp = ctx.enter_context(tc.tile_pool(name="sp", bufs=2, space="PSUM"))
    ot = ctx.enter_context(tc.tile_pool(name="pot", bufs=4, space="PSUM"))

    # ---- tiny consts ----
    zcol = pc.tile([128, 1], f32, tag="zcol", name="zcol")
    nc.vector.memset(zcol[:], 0.0)
    nc.const_aps.aps[(f32, 0.0)] = zcol[:]
    ones4 = pc.tile([128, 2, 1], fp8, tag="ones4", name="ones4")
    nc.vector.memset(ones4[:], 1.0)
    ones5 = pc.tile([128, 2, 1], fp8e5, tag="ones5", name="ones5")
    nc.vector.memset(ones5[:], 1.0)

    # ---- PE warmup: dep-free matmuls bridge the DMA head + pstate ramp
    wdum = pc.tile([128, 128], bf16, tag="wdum", name="wdum")
    nc.vector.memset(wdum[:], 1.0)
    wslot = ot.tile([128, 512], f32, tag="warm", name="warm")
    for i in range(N_WARMUP):
        nc.tensor.matmul(wslot[:, 0:128], wdum[:], wdum[:],
                         start=True, stop=True)

    # ---- input DMAs: the DMA fabric is serial -- order by need.
    # xTw8[p, pr, t, kt, j] = x[t*128+j, (2*pr+kt)*128+p], in quarters
    xTw8 = pb.tile([128, 16, 2, 2, 128], fp8, tag="xTw8", name="xTw8")
    xTw8f = xTw8[:].rearrange("p a b c e -> p (a b c e)")
    qs = [nc.sync, nc.scalar]
    cpack = pc.tile([128, CPW], f32, tag="cpack", name="cpack")
    for qr in range(4):
        qs[qr % 2].dma_start(xTw8f[:, qr * 2048:(qr + 1) * 2048],
                             d["xTw8"][:, qr * 2048:(qr + 1) * 2048])
        if qr == 1:
            nc.sync.dma_start(cpack[:], d["cpack"][:])
    # wb[p, oh, kt, j] = w0^T[kt*128+p, oh*128+j]
    wb = pb.tile([128, 2, 2, 128], bf16, tag="wb", name="wb")
    nc.sync.dma_start(wb[:].rearrange("p a b c -> p (a b c)"), d["wb"][:])
    # w1p8[p, cs, kto, j] = w1[kto*128+p, cs*128+j]  (plain w1, fp8)
    w1p8 = pb.tile([128, 2, 2, 128], fp8, tag="w1p8", name="w1p8")
    nc.scalar.dma_start(w1p8[:].rearrange("p a b c -> p (a b c)"), d["w1p8"][:])
    # w23t[p, oh, kt, j] = (w3 w2)^T[kt*128+p, oh*128+j]
    w23t = pb.tile([128, 2, 2, 128], bf16, tag="w23t", name="w23t")
    nc.sync.dma_start(w23t[:].rearrange("p a b c -> p (a b c)"), d["w23t"][:])
    # xs8[p, mt, kt, j] = x[kt*128+p, mt*128+j], halves
    xs8 = pb.tile([128, MT, 2, 128], fp8, tag="xs8", name="xs8")
    xs8f = xs8[:].rearrange("p a b c -> p (a b c)")
    nc.scalar.dma_start(xs8f[:, 0:4096], d["xs8"][:, 0:4096])
    nc.sync.dma_start(xs8f[:, 4096:8192], d["xs8"][:, 4096:8192])
    # xq (f32 residual) is emitted LAST -- only needed by the final stt

    cvec = [cpack[:, CV0:CV0 + 8], cpack[:, CV1:CV1 + 8]]
    gm = [cpack[:, GMA:GMA + 32], cpack[:, GMB:GMB + 32]]
    gmt = [cpack[0:32, GTA:GTA + 128], cpack[0:32, GTB:GTB + 128]]
    ident = cpack[:, IDT:IDT + 128]
    dmask = [cpack[:, IDT:IDT + 256], cpack[:, MZL:MZL + 256]]
    identw = cpack[:, IDW:IDW + 128]
    epscol = cpack[0:32, EPC:EPC + 1]

    # ---- GroupNorm stats via PE Gram over xTw8 ----
    # gtile: [t0 gram 0:256 | t0 sum-x 256 | pad | t1 gram 512:768 | t1 sum-x]
    gtile = sp.tile([128, 1024], f32, tag="sp", name="gram")
    goff = [0, 512]
    for pr in range(16):
        for t in range(2):
            lhs = xTw8[:, pr, t]
            nc.tensor.matmul(
                gtile[:, goff[t]:goff[t] + 256],
                lhs,
                xTw8[:, pr].rearrange("p t kt j -> p kt t j"),
                start=(pr == 0), stop=(pr == 15), perf_mode=DR,
            )
            nc.tensor.matmul(
                gtile[:, goff[t] + 256:goff[t] + 257],
                lhs, ones4[:],
                start=(pr == 0), stop=(pr == 15), perf_mode=DR,
            )
    # pt[t]: col0 = sum x, col1 = sum x^2 (diag extract)
    pt = [ptiny.tile([128, 2], f32, tag=f"pt{t}", name=f"pt{t}") for t in range(2)]
    trash = pw.tile([128, 256], f32, tag="trash", name="trash", bufs=2)
    for t in range(2):
        nc.vector.tensor_copy(pt[t][:, 0:1], gtile[:, goff[t] + 256:goff[t] + 257])
        nc.vector.scalar_tensor_tensor(
            trash[:], gtile[:, goff[t]:goff[t] + 256], 1.0, dmask[t],
            op0=ALU.mult, op1=ALU.mult, accum_out=pt[t][:, 1:2],
        )

    # group stats -> mu/rstd -> per-channel a, bfold
    s32 = ot.tile([128, 512], f32, tag="warm", name="s32")
    for t in range(2):
        nc.tensor.matmul(s32[0:32, 0:2], gm[t], pt[t][:],
                         start=(t == 0), stop=(t == 1))
    sg = ptiny.tile([32, 2], f32, tag="sg", name="sg")
    nc.vector.tensor_copy(sg[:], s32[0:32, 0:2])
    mr = ptiny.tile([32, 2], f32, tag="mr", name="mr")
    musq = ptiny.tile([32, 1], f32, tag="musq", name="musq")
    nc.vector.tensor_mul(musq[:], sg[:, 0:1], sg[:, 0:1])
    var = ptiny.tile([32, 1], f32, tag="var", name="var")
    nc.vector.tensor_sub(var[:], sg[:, 1:2], musq[:])
    std = ptiny.tile([32, 1], f32, tag="std", name="std")
    nc.scalar.activation(std[:], var[:], AF.Sqrt, bias=epscol)
    # dummy Exp preloads the exp table during the DMA head
    edum = ptiny.tile([128, 1], u8, tag="edum", name="edum")
    nc.scalar.activation(edum[:].bitcast(fp8e5), zcol[:], AF.Exp)
    nc.vector.reciprocal(mr[:, 1:2], std[:])
    nc.vector.tensor_mul(mr[:, 0:1], sg[:, 0:1], mr[:, 1:2])

    # gmaskT carries gamma (host-folded): bc = [mu*rstd*gamma, rstd*gamma=a]
    a_t, bfold_bf = [], []
    bc = ot.tile([128, 512], f32, tag="warm", name="bc")
    for t in range(2):
        nc.tensor.matmul(bc[:, 2 * t:2 * t + 2], gmt[t], mr[:],
                         start=True, stop=True)
    for t in range(2):
        a = pb.tile([128, 1], f32, tag=f"a{t}", name=f"a{t}")
        nc.vector.tensor_copy(a[:], bc[:, 2 * t + 1:2 * t + 2])
        bb = pb.tile([128, 1], bf16, tag=f"bfb{t}", name=f"bfb{t}")
        nc.vector.tensor_scalar(bb[:], bc[:, 2 * t:2 * t + 1], -1.0,
                                cvec[t][:, 1:2], op0=ALU.mult, op1=ALU.add)
        a_t.append(a)
        bfold_bf.append(bb)

    def _early_out(srcs):
        for oh in range(2):
            for ch in range(2):
                yt = pw.tile([128, 512], f32, tag="yt", name="yt")
                nc.vector.tensor_copy(
                    yt[:], srcs[oh][:, ch * 512:(ch + 1) * 512])
                nc.sync.dma_start(
                    d["y"][:, oh * NSH + ch * 512:oh * NSH + (ch + 1) * 512],
                    yt[:])

    # ---- effective biases (RAW weights -- emitted before the a-fold) ----
    beff0 = []
    for oh in range(2):
        bp = ot.tile([128, 512], f32, tag="warm", name="bp")
        for t in range(2):
            nc.tensor.matmul(bp[:, 0:1], wb[:, oh, t],
                             bfold_bf[t][:], start=(t == 0), stop=(t == 1))
        bs = pb.tile([128, 1], f32, tag=f"beff0_{oh}", name=f"beff0_{oh}")
        nc.scalar.activation(bs[:], bp[:, 0:1], AF.Identity,
                             bias=cvec[oh][:, 2:3])
        beff0.append(bs)
    b3eff = []
    for oh in range(2):
        bp = ot.tile([128, 512], f32, tag="warm", name="bp3")
        for t in range(2):
            nc.tensor.matmul(bp[:, 0:1], w23t[:, oh, t],
                             bfold_bf[t][:], start=(t == 0), stop=(t == 1))
        bs = pb.tile([128, 1], f32, tag=f"b3eff{oh}", name=f"b3eff{oh}")
        nc.scalar.activation(bs[:], bp[:, 0:1], AF.Identity,
                             bias=cvec[oh][:, 3:4])
        b3eff.append(bs)

    # xqb = x-shard + b3eff (f32); the final stt adds proj*2^-19 onto it.
    # xq is the last input DMA issued -- its transfer queues behind all the
    # early-needed inputs on the serial DMA fabric but lands well before the
    # first finish phase needs xqb.
    xq = pb.tile([128, 2, NSH], f32, tag="xq", name="xq")
    nc.sync.dma_start(xq[:].rearrange("p a b -> p (a b)"), d["xq"][:])
    xqb = pb.tile([128, 2, NSH], f32, tag="xqb", name="xqb")

    def emit_xqb(t):
        # on gpsimd: its queue is idle, so blocking on the late xq DMA is
        # free (on DVE the scheduler head-of-line-blocked the queue)
        nc.gpsimd.tensor_scalar(xqb[:, t], xq[:, t], b3eff[t][:], None,
                                op0=ALU.add)

    # ---- fold a into the fp8 weights directly (one DVE op each) ----
    w018 = pb.tile([128, 2, 2, 128], fp8, tag="w018", name="w018")
    w238 = pb.tile([128, 2, 2, 128], fp8, tag="w238", name="w238")
    for t in range(2):
        nc.vector.tensor_scalar_mul(w018[:, :, t], wb[:, :, t], a_t[t][:])
        nc.vector.tensor_scalar(w238[:, :, t], w23t[:, :, t], a_t[t][:], WS,
                                op0=ALU.mult, op1=ALU.mult)

    if PHASE <= 1:
        _early_out([xq[:, 0], xq[:, 1]])
        ctx.close()
        return

    # ---- q = w0a @ xq + beff0 : fp8 [128, 2(kt=oh), NSH] ----
    # q and g run through the 4-slot ot pool so the 2 big sp slots stay free
    # for the attention pipeline (the sp ring was serializing q -> g -> S).
    q2 = pb.tile([128, 2, NSH], fp8, tag="q2", name="q2")
    for oh in range(2):
        for ch in range(2):
            qp = ot.tile([128, 512], f32, tag="warm", name=f"qp{oh}{ch}")
            nc.tensor.matmul(
                qp[:],
                w018[:, oh],
                xs8[:, ch * 4:(ch + 1) * 4].rearrange(
                    "p mt kt j -> p kt mt j"),
                start=True, stop=True, perf_mode=DR,
            )
            dst = q2[:, oh, ch * 512:(ch + 1) * 512]
            if (oh + ch) % 2 == 0:
                nc.scalar.activation(dst, qp[:], AF.Identity,
                                     bias=beff0[oh][:])
            else:
                nc.vector.tensor_scalar(dst, qp[:], beff0[oh][:], None,
                                        op0=ALU.add)

    # ---- g = a * (w1^T q) : fp8 [128, 2(kt=c-slice), NSH] ----
    g8 = pb.tile([128, 2, NSH], fp8, tag="g8", name="g8")
    for h in range(2):
        for cs in range(2):
            gp = ot.tile([128, 512], f32, tag="warm", name=f"gp{cs}{h}")
            nc.tensor.matmul(
                gp[:],
                w1p8[:, cs],
                q2[:, :, h * 512:(h + 1) * 512],
                start=True, stop=True, perf_mode=DR,
            )
            dst = g8[:, cs, h * 512:(h + 1) * 512]
            if (cs + h) % 2 == 0:
                nc.scalar.activation(dst, gp[:], AF.Copy, scale=a_t[cs][:])
            else:
                nc.vector.tensor_scalar_mul(dst, gp[:], a_t[cs][:])

    if PHASE == 2:
        _early_out([xq[:, 0], xq[:, 1]])
        ctx.close()
        return

    # ---- attention ----
    yts = [pb.tile([128, NSH], f32, tag=f"yts{t}", name=f"yts{t}")
           for t in range(2)]
    # attnx8[p, t(kt for proj), half, n] -- normalized attention-averaged x
    attnx8 = pb.tile([128, 2, 2, 512], fp8, tag="attnx8", name="attnx8")

    def emit_s_exp(half, pr):
        # S^T pair: S[m, n] = sum_c x[c, m] g[c, n], then exp -> E chunk
        st = sp.tile([128, 1024], f32, tag="sp", name="st")
        for h in range(2):
            nc.tensor.matmul(
                st[:, h * 512:(h + 1) * 512],
                xs8[:, 2 * pr + h],
                g8[:, :, half * 512:(half + 1) * 512],
                start=True, stop=True, perf_mode=DR,
            )
        # exp -> fp8e5 E chunks, permuted out to [p, ns, kt(2 mt), j].
        # Separate tiles per engine: co-writing one tile through permuted
        # views serializes ACT->DVE in the scheduler.
        stv = st[:].rearrange("p (kt ns j) -> p kt ns j", kt=2, ns=4)
        e1 = pw.tile([128, 2, 2, 128], u8, tag="e1", name="e1", bufs=5)
        e2 = pw.tile([128, 2, 2, 128], u8, tag="e2", name="e2", bufs=5)
        ev1 = e1[:].rearrange("p ns kt j -> p kt ns j")
        ev2 = e2[:].rearrange("p ns kt j -> p kt ns j")
        if EXP_SPLIT[half * 16 + pr] >= 2:
            nc.scalar.activation(ev1.bitcast(fp8e5), stv[:, :, 0:2, :],
                                 AF.Exp, scale=SCALE)
            nc.vector.tensor_scalar(ev2, stv[:, :, 2:4, :],
                                    SCALE * 4.0 * LOG2E, 60.0,
                                    op0=ALU.mult, op1=ALU.add)
        else:
            nc.vector.tensor_scalar(ev1, stv[:, :, 0:2, :],
                                    SCALE * 4.0 * LOG2E, 60.0,
                                    op0=ALU.mult, op1=ALU.add)
            nc.scalar.activation(ev2.bitcast(fp8e5), stv[:, :, 2:4, :],
                                 AF.Exp, scale=SCALE)
        return (e1, e2)

    def emit_avx(half, pr, e, ots):
        # AVx accumulation + denominator columns (in ots[0] cols 256:260)
        e1, e2 = e
        xr = xTw8[:, pr].rearrange("p t kt j -> p kt t j")
        for ns in range(4):
            et = e1 if ns < 2 else e2
            el = et[:, ns % 2].bitcast(fp8e5)
            nc.tensor.matmul(
                ots[ns][:, 0:256], el, xr,
                start=(pr == 0), stop=(pr == 15), perf_mode=DR,
            )
            nc.tensor.matmul(
                ots[0][:, 256 + ns:257 + ns], el, ones5[:],
                start=(pr == 0), stop=(pr == 15), perf_mode=DR,
            )


    def finish_steps(half, ots, use_sp=False):
        # generator of emission steps; each step is interleaved between the
        # next half's pairs so no engine queue sees a serial finish block.
        rec4 = ptiny.tile([128, 4], f32, tag="rec4", name="rec4")
        nc.vector.reciprocal(rec4[:], ots[0][:, 256:260])
        trps = [None, None]
        ons = []

        def emit_on(ns):
            on = pw.tile([128, 256], f32, tag="on", name="on", bufs=5)
            if ON_ACT[half * 4 + ns]:
                nc.scalar.activation(on[:], ots[ns][:, 0:256], AF.Identity,
                                     scale=rec4[:, ns:ns + 1])
            else:
                nc.vector.tensor_scalar_mul(on[:], ots[ns][:, 0:256],
                                            rec4[:, ns:ns + 1])
            ons.append(on)
            if ns == 0:
                trps[0] = ot.tile([128, 512], f32, tag="warm",
                                  name=f"trp{half}_0")[:]
                trps[1] = ot.tile([128, 512], f32, tag="warm",
                                  name=f"trp{half}_1")[:]
            for t in range(2):
                nc.tensor.transpose(trps[t][:, ns * 128:(ns + 1) * 128],
                                    ons[ns][:, t * 128:(t + 1) * 128], ident)

        def emit_drain(t):
            dst = attnx8[:, t, half]
            if TRP_ACT[half * 2 + t]:
                nc.scalar.activation(dst, trps[t], AF.Copy)
            else:
                nc.vector.tensor_copy(dst, trps[t])

        pjs = [None, None]

        def emit_preload(oh):
            # residual preload: exact fp32 identity matmul of xqb*WS into the
            # projection psum; the DR projection then accumulates on top.
            pj = ot.tile([128, 512], f32, tag="warm",
                         name=f"pj{half}_{oh}")[:]
            pjs[oh] = pj
            nc.tensor.matmul(pj, identw,
                             xqb[:, oh, half * 512:(half + 1) * 512],
                             start=True, stop=False, skip_group_check=True)

        def emit_proj(oh):
            pj = pjs[oh]
            nc.tensor.matmul(pj, w238[:, oh], attnx8[:, :, half],
                             start=False, stop=True, perf_mode=DR,
                             skip_group_check=True)
            sl = yts[oh][:, half * 512:(half + 1) * 512]
            if (half + oh) % 2 == 0:
                nc.scalar.activation(sl, pj, AF.Copy, scale=1.0 / WS)
            else:
                nc.vector.tensor_scalar_mul(sl, pj, 1.0 / WS)
            dq = (nc.sync, nc.scalar)[(half + oh) % 2]
            dq.dma_start(
                d["y"][:, oh * NSH + half * 512:oh * NSH + (half + 1) * 512],
                sl,
            )

        def emit_proj_stt(oh):
            # tail half: DVE is free -> plain projection + stt keeps the
            # in-order PE stream short.
            pj = ot.tile([128, 512], f32, tag="warm",
                         name=f"pj{half}_{oh}")[:]
            nc.tensor.matmul(pj, w238[:, oh], attnx8[:, :, half],
                             start=True, stop=True, perf_mode=DR)
            sl = yts[oh][:, half * 512:(half + 1) * 512]
            nc.vector.scalar_tensor_tensor(
                sl, pj, 1.0 / WS, xqb[:, oh, half * 512:(half + 1) * 512],
                op0=ALU.mult, op1=ALU.add,
            )
            dq = (nc.sync, nc.scalar)[(half + oh) % 2]
            dq.dma_start(
                d["y"][:, oh * NSH + half * 512:oh * NSH + (half + 1) * 512],
                sl,
            )

        yield lambda: emit_on(0)
        yield lambda: emit_on(1)
        yield lambda: emit_on(2)
        yield lambda: emit_on(3)
        yield lambda: emit_drain(0)
        yield lambda: emit_drain(1)
        if not use_sp:
            yield lambda: emit_preload(0)
            yield lambda: emit_preload(1)
            yield lambda: emit_proj(0)
            yield lambda: emit_proj(1)
        else:
            yield lambda: emit_proj_stt(0)
            yield lambda: emit_proj_stt(1)

    pending = iter([lambda: emit_xqb(0), lambda: emit_xqb(1)])
    for half in range(2):
        ots = [ot.tile([128, 512], f32, tag="warm", name=f"ot{half}_{ns}")
               for ns in range(4)]
        e_cur = emit_s_exp(half, 0)
        for pr in range(16):
            e_nxt = emit_s_exp(half, pr + 1) if pr + 1 < 16 else None
            emit_avx(half, pr, e_cur, ots)
            e_cur = e_nxt
            if pending is not None:
                nxt = next(pending, None)
                if nxt is None:
                    pending = None
                else:
                    nxt()
        pending = finish_steps(half, ots, use_sp=(half == 1))
    for step in pending:
        step()

    ctx.close()


_CACHE = {}


def _get_program():
    if "nc" in _CACHE:
        return _CACHE["nc"], _CACHE["dram"]
    nc = bacc.Bacc("TRN2", target_bir_lowering=False, debug=False,
                   enable_asserts=False, num_devices=NCORES)
    d = {}
    d["xs8"] = nc.dram_tensor("xs8", [128, MT * 256], fp8,
                              kind="ExternalInput").ap()
    d["xTw8"] = nc.dram_tensor("xTw8", [128, 16 * 512], fp8,
                               kind="ExternalInput").ap()
    d["xq"] = nc.dram_tensor("xq", [128, 2 * NSH], f32, kind="ExternalInput").ap()
    d["wb"] = nc.dram_tensor("wb", [128, 4 * 128], bf16, kind="ExternalInput").ap()
    d["w1p8"] = nc.dram_tensor("w1p8", [128, 4 * 128], fp8,
                               kind="ExternalInput").ap()
    d["w23t"] = nc.dram_tensor("w23t", [128, 4 * 128], bf16,
                               kind="ExternalInput").ap()
    d["cpack"] = nc.dram_tensor("cpack", [128, CPW], f32,
                                kind="ExternalInput").ap()
    d["y"] = nc.dram_tensor("y", [128, 2 * NSH], f32, kind="ExternalOutput").ap()

    with tile.TileContext(nc) as tc:
        _build_body(nc, tc, d)
    nc.compile()
    _CACHE["nc"] = nc
    _CACHE["dram"] = d
    return nc, d


def make_in_maps(x, gamma, beta, w0, b0, w1, b1, w2, b2, w3, b3):
    """Host-side sharding/layout prep: returns list of 8 per-core inputs."""
    e4 = ml_dtypes.float8_e4m3
    xb = np.ascontiguousarray(np.asarray(x, np.float32).reshape(B, C, N))

    cpack = np.zeros((128, CPW), np.float32)
    gamma = np.asarray(gamma, np.float32)
    beta = np.asarray(beta, np.float32)
    b0 = np.asarray(b0, np.float32)
    bout = (np.asarray(w3, np.float32) @ np.asarray(b2, np.float32)
            + np.asarray(b3, np.float32))
    for t, off in ((0, CV0), (1, CV1)):
        sl = slice(t * 128, (t + 1) * 128)
        cpack[:, off + 0] = gamma[sl]
        cpack[:, off + 1] = beta[sl]
        cpack[:, off + 2] = b0[sl]
        cpack[:, off + 3] = bout[sl]
        cpack[:, off + 4] = -gamma[sl]
    for t, off in ((0, GMA), (1, GMB)):
        ch = t * 128 + np.arange(128)
        cpack[np.arange(128), off + ch // CPG] = 1.0 / NPG
    for t, off in ((0, GTA), (1, GTB)):
        ch = t * 128 + np.arange(128)
        cpack[ch // CPG, off + np.arange(128)] = gamma[ch]
    cpack[:, IDT:IDT + 128] = np.eye(128, dtype=np.float32)
    cpack[:, IDW:IDW + 128] = np.eye(128, dtype=np.float32) * WS
    cpack[:, EPC] = EPS

    # wb[p, oh, kt, j] = w0^T[kt*128+p, oh*128+j]
    w0t = np.asarray(w0, np.float32).T.reshape(2, 128, 2, 128)  # [kt, p, oh, j]
    wb = w0t.transpose(1, 2, 0, 3).reshape(128, -1).astype(ml_dtypes.bfloat16)
    # w1p8[p, cs, kto, j] = w1[kto*128+p, cs*128+j]
    w1a = np.asarray(w1, np.float32).reshape(2, 128, 2, 128)    # [kto, p, cs, j]
    w1p8 = w1a.transpose(1, 2, 0, 3).reshape(128, -1).astype(e4)
    w23 = (np.asarray(w3, np.float32) @ np.asarray(w2, np.float32)).T
    w23t = w23.reshape(2, 128, 2, 128).transpose(1, 2, 0, 3)
    w23t = w23t.reshape(128, -1).astype(ml_dtypes.bfloat16)

    in_maps = []
    for core in range(NCORES):
        b, j = divmod(core, 4)
        # rotate the token blocks so this core's query shard is mt 0..7;
        # attention sums over m, so any consistent xs8/xTw8 order works
        rot = np.roll(np.arange(MT), -j * 8)
        xc = xb[b]
        xs8 = xc.reshape(2, 128, MT, 128).transpose(1, 2, 0, 3)[:, rot]
        xT = xc.reshape(2, 128, 16, 2, 128)  # [t, jj, pr, kt, p]
        xT = xT.transpose(4, 2, 3, 0, 1).reshape(128, MT, 2, 128)
        xT = xT[:, rot].reshape(128, 16, 2, 2, 128).transpose(0, 1, 3, 2, 4)
        xqc = xc[:, j * NSH:(j + 1) * NSH]
        xq = xqc.reshape(2, 128, NSH).transpose(1, 0, 2).reshape(128, -1)
        m = {
            "xs8": xs8.reshape(128, -1).astype(e4),
            "xTw8": np.ascontiguousarray(xT).reshape(128, -1).astype(e4),
            "xq": np.ascontiguousarray(xq),
            "wb": wb, "w1p8": w1p8, "w23t": w23t, "cpack": cpack,
        }
        in_maps.append(m)
    return in_maps


def assemble_output(results):
    out = np.zeros((B, C, N), np.float32)
    for core in range(NCORES):
        b, j = divmod(core, 4)
        y = results[core]["y"].reshape(128, 2, NSH).transpose(1, 0, 2)
        out[b][:, j * NSH:(j + 1) * NSH] = y.reshape(C, NSH)
    return out.reshape(B, C, 16, 16, 16)


def kernel(x, gamma, beta, w0, b0, w1, b1, w2, b2, w3, b3):
    nc, _ = _get_program()
    in_maps = make_in_maps(x, gamma, beta, w0, b0, w1, b1, w2, b2, w3, b3)
    res = bass_utils.run_bass_kernel_spmd(nc, in_maps, core_ids=list(range(NCORES)))
    return assemble_output(res.results)


# revision 31
# speedup vs baseline: 1.1142x; 1.0267x over previous
"""Trainium2 Bass/Tile kernel for AttnBlock:
GroupNorm(32) -> 1x1 conv q,k,v -> softmax attention over N=4096 tokens
-> 1x1 conv proj -> residual.

Sharding: 8 cores = 2 (batch) x 4 (query-token shards of N).  Each core gets
the full x of its batch plus its n-shard slice, and produces the [C, N/4]
output shard.  No collectives.

Architecture (v4):
- All heavy matmuls are fp8 MatmulPerfMode.DoubleRow: the full K=256
  contraction in one instruction at 0.5 cycles/output-column.  DR stationary
  operands need their 256 weight elements contiguous per partition; every
  lhsT is laid out [.., kt(2), 128].
- GroupNorm stats via a PE Gram-matrix over the m-major fp8 x copy
  (diag -> sum x^2, ones-matmul -> sum x), diag extracted by one DVE
  scalar_tensor_tensor+accum per c-tile.
- No k tensor: S^T = x^T g with g = a*(w1^T q) [C, NSH] -- the PSUM->SBUF
  drain is the n-shard-sized g (2K lanes) instead of the m-sized k (8K).
  The k bias is dropped exactly (softmax shift invariance); q keeps its
  effective bias.
- No v tensor: attention accumulates over x itself:
  AVx[n, c] = sum_m E[m, n] x[c, m] (moving operand = resident xTw8),
  plus denominator columns from a tiny ones matmul per ns.  After
  normalize + transpose, ONE DoubleRow projection by w238 = a*(w3 w2)^T
  (host-folded w3@w2, scaled 2^19 for fp8) produces the output; the scale
  is undone in the final scalar_tensor_tensor against xqb = x + b3eff.
- Softmax over 2-bank [128,1024] S^T psum tiles; exp ns-subtiles split
  between ACT (true Exp -> fp8e5) and DVE (Schraudolph bits =
  round(logit*4*log2e + 60) as uint8 == fp8e5m2; e5m2 because logits span
  +-8).  Output APs are permuted so E tiles come out [ns, kt, j] -- the
  DR lhsT layout for AVx.
"""

import ml_dtypes
import numpy as np

import concourse.bacc as bacc
import concourse.mybir as mybir
import concourse.tile as tile
from concourse import bass_utils

f32 = mybir.dt.float32
bf16 = mybir.dt.bfloat16
fp8 = mybir.dt.float8e4
fp8e5 = mybir.dt.float8e5
u8 = mybir.dt.uint8
AF = mybir.ActivationFunctionType
ALU = mybir.AluOpType
DR = mybir.MatmulPerfMode.DoubleRow

B = 2
C = 256
N = 4096          # 16**3 tokens
NSH = N // 4      # 1024 tokens per core
G = 32
CPG = C // G      # channels per group
NPG = CPG * N     # elements per group
EPS = 1e-6
SCALE = C ** -0.5          # 1/16
LOG2E = float(1.0 / np.log(2.0))
WS = 524288.0              # 2^19 fp8-range scale on w23; undone in the stt
MT = N // 128              # 32 m-tiles

NCORES = 8

# cpack column layout
CV0, CV1 = 0, 8            # cvec slice0/1: [gamma, beta, b0, bout, -gamma]
GMA, GMB = 16, 48          # gmask per slice [128, 32] (1/NPG folded)
GTA, GTB = 80, 208         # gmaskT per slice [32, 128] on partitions 0:32
MZL = 336                  # zeros[128] | ident[128] | zeros[128]
IDT = 464
EPC = 720                  # eps column
IDW = 728                  # identity * WS (residual preload)
CPW = 856

# engine splits (True -> ACT, False -> DVE)
EXP_SPLIT = [2] * 24 + [3] * 8  # ns-subtiles on ACT per (half*16+pair)
GEP_ACT = [True, False]    # g drain per c-slice
ON_ACT = [True, False, True, False] * 2  # normalize per (half*4 + ns)
TRP_ACT = [True, False, True, False]  # attnx drain per (half*2 + t)

N_WARMUP = 42
EBUFS = 8
PHASE = 4


def _build_body(nc, tc, d):
    from contextlib import ExitStack

    ctx = ExitStack()
    pc = ctx.enter_context(tc.tile_pool(name="const", bufs=1))
    pb = ctx.enter_context(tc.tile_pool(name="big", bufs=1))
    pw = ctx.enter_context(tc.tile_pool(name="work", bufs=3))
    ptiny = ctx.enter_context(tc.tile_pool(name="tiny", bufs=2))
    # PSUM: sp = 2 x [128,1024] (2 banks each), ot = 4 x [128,512] (1 bank)
    sp = ctx.enter_context(tc.tile_pool(name="sp", bufs=2, space="PSUM"))
    ot = ctx.enter_context(tc.tile_pool(name="pot", bufs=4, space="PSUM"))

    # ---- tiny consts ----
    zcol = pc.tile([128, 1], f32, tag="zcol", name="zcol")
    nc.vector.memset(zcol[:], 0.0)
    nc.const_aps.aps[(f32, 0.0)] = zcol[:]
    ones4 = pc.tile([128, 2, 1], fp8, tag="ones4", name="ones4")
    nc.vector.memset(ones4[:], 1.0)
    ones5 = pc.tile([128, 2, 1], fp8e5, tag="ones5", name="ones5")
    nc.vector.memset(ones5[:], 1.0)

    # ---- PE warmup: dep-free matmuls bridge the DMA head + pstate ramp
    wdum = pc.tile([128, 128], bf16, tag="wdum", name="wdum")
    nc.vector.memset(wdum[:], 1.0)
    wslot = ot.tile([128, 512], f32, tag="warm", name="warm")
    for i in range(N_WARMUP):
        nc.tensor.matmul(wslot[:, 0:128], wdum[:], wdum[:],
                         start=True, stop=True)

    # ---- input DMAs: the DMA fabric is serial -- order by need.
    # xTw8[p, pr, t, kt, j] = x[t*128+j, (2*pr+kt)*128+p], in quarters
    xTw8 = pb.tile([128, 16, 2, 2, 128], fp8, tag="xTw8", name="xTw8")
    xTw8f = xTw8[:].rearrange("p a b c e -> p (a b c e)")
    qs = [nc.sync, nc.scalar]
    cpack = pc.tile([128, CPW], f32, tag="cpack", name="cpack")
    for qr in range(4):
        qs[qr % 2].dma_start(xTw8f[:, qr * 2048:(qr + 1) * 2048],
                             d["xTw8"][:, qr * 2048:(qr + 1) * 2048])
        if qr == 1:
            nc.sync.dma_start(cpack[:], d["cpack"][:])
    # wb[p, oh, kt, j] = w0^T[kt*128+p, oh*128+j]
    wb = pb.tile([128, 2, 2, 128], bf16, tag="wb", name="wb")
    nc.sync.dma_start(wb[:].rearrange("p a b c -> p (a b c)"), d["wb"][:])
    # w1p8[p, cs, kto, j] = w1[kto*128+p, cs*128+j]  (plain w1, fp8)
    w1p8 = pb.tile([128, 2, 2, 128], fp8, tag="w1p8", name="w1p8")
    nc.scalar.dma_start(w1p8[:].rearrange("p a b c -> p (a b c)"), d["w1p8"][:])
    # w23t[p, oh, kt, j] = (w3 w2)^T[kt*128+p, oh*128+j]
    w23t = pb.tile([128, 2, 2, 128], bf16, tag="w23t", name="w23t")
    nc.sync.dma_start(w23t[:].rearrange("p a b c -> p (a b c)"), d["w23t"][:])
    # xs8[p, mt, kt, j] = x[kt*128+p, mt*128+j], halves
    xs8 = pb.tile([128, MT, 2, 128], fp8, tag="xs8", name="xs8")
    xs8f = xs8[:].rearrange("p a b c -> p (a b c)")
    nc.scalar.dma_start(xs8f[:, 0:4096], d["xs8"][:, 0:4096])
    nc.sync.dma_start(xs8f[:, 4096:8192], d["xs8"][:, 4096:8192])
    # xq (f32 residual) is emitted LAST -- only needed by the final stt

    cvec = [cpack[:, CV0:CV0 + 8], cpack[:, CV1:CV1 + 8]]
    gm = [cpack[:, GMA:GMA + 32], cpack[:, GMB:GMB + 32]]
    gmt = [cpack[0:32, GTA:GTA + 128], cpack[0:32, GTB:GTB + 128]]
    ident = cpack[:, IDT:IDT + 128]
    dmask = [cpack[:, IDT:IDT + 256], cpack[:, MZL:MZL + 256]]
    identw = cpack[:, IDW:IDW + 128]
    epscol = cpack[0:32, EPC:EPC + 1]

    # ---- GroupNorm stats via PE Gram over xTw8 ----
    # gtile: [t0 gram 0:256 | t0 sum-x 256 | pad | t1 gram 512:768 | t1 sum-x]
    gtile = sp.tile([128, 1024], f32, tag="sp", name="gram")
    goff = [0, 512]
    for pr in range(16):
        for t in range(2):
            lhs = xTw8[:, pr, t]
            nc.tensor.matmul(
                gtile[:, goff[t]:goff[t] + 256],
                lhs,
                xTw8[:, pr].rearrange("p t kt j -> p kt t j"),
                start=(pr == 0), stop=(pr == 15), perf_mode=DR,
            )
            nc.tensor.matmul(
                gtile[:, goff[t] + 256:goff[t] + 257],
                lhs, ones4[:],
                start=(pr == 0), stop=(pr == 15), perf_mode=DR,
            )
    # pt[t]: col0 = sum x, col1 = sum x^2 (diag extract)
    pt = [ptiny.tile([128, 2], f32, tag=f"pt{t}", name=f"pt{t}") for t in range(2)]
    trash = pw.tile([128, 256], f32, tag="trash", name="trash", bufs=2)
    for t in range(2):
        nc.vector.tensor_copy(pt[t][:, 0:1], gtile[:, goff[t] + 256:goff[t] + 257])
        nc.vector.scalar_tensor_tensor(
            trash[:], gtile[:, goff[t]:goff[t] + 256], 1.0, dmask[t],
            op0=ALU.mult, op1=ALU.mult, accum_out=pt[t][:, 1:2],
        )

    # group stats -> mu/rstd -> per-channel a, bfold
    s32 = ot.tile([128, 512], f32, tag="warm", name="s32")
    for t in range(2):
        nc.tensor.matmul(s32[0:32, 0:2], gm[t], pt[t][:],
                         start=(t == 0), stop=(t == 1))
    sg = ptiny.tile([32, 2], f32, tag="sg", name="sg")
    nc.vector.tensor_copy(sg[:], s32[0:32, 0:2])
    mr = ptiny.tile([32, 2], f32, tag="mr", name="mr")
    musq = ptiny.tile([32, 1], f32, tag="musq", name="musq")
    nc.vector.tensor_mul(musq[:], sg[:, 0:1], sg[:, 0:1])
    var = ptiny.tile([32, 1], f32, tag="var", name="var")
    nc.vector.tensor_sub(var[:], sg[:, 1:2], musq[:])
    std = ptiny.tile([32, 1], f32, tag="std", name="std")
    nc.scalar.activation(std[:], var[:], AF.Sqrt, bias=epscol)
    # dummy Exp preloads the exp table during the DMA head
    edum = ptiny.tile([128, 1], u8, tag="edum", name="edum")
    nc.scalar.activation(edum[:].bitcast(fp8e5), zcol[:], AF.Exp)
    nc.vector.reciprocal(mr[:, 1:2], std[:])
    nc.vector.tensor_mul(mr[:, 0:1], sg[:, 0:1], mr[:, 1:2])

    # gmaskT carries gamma (host-folded): bc = [mu*rstd*gamma, rstd*gamma=a]
    a_t, bfold_bf = [], []
    bc = ot.tile([128, 512], f32, tag="warm", name="bc")
    for t in range(2):
        nc.tensor.matmul(bc[:, 2 * t:2 * t + 2], gmt[t], mr[:],
                         start=True, stop=True)
    for t in range(2):
        a = pb.tile([128, 1], f32, tag=f"a{t}", name=f"a{t}")
        nc.vector.tensor_copy(a[:], bc[:, 2 * t + 1:2 * t + 2])
        bb = pb.tile([128, 1], bf16, tag=f"bfb{t}", name=f"bfb{t}")
        nc.vector.tensor_scalar(bb[:], bc[:, 2 * t:2 * t + 1], -1.0,
                                cvec[t][:, 1:2], op0=ALU.mult, op1=ALU.add)
        a_t.append(a)
        bfold_bf.append(bb)

    def _early_out(srcs):
        for oh in range(2):
            for ch in range(2):
                yt = pw.tile([128, 512], f32, tag="yt", name="yt")
                nc.vector.tensor_copy(
                    yt[:], srcs[oh][:, ch * 512:(ch + 1) * 512])
                nc.sync.dma_start(
                    d["y"][:, oh * NSH + ch * 512:oh * NSH + (ch + 1) * 512],
                    yt[:])

    # ---- effective biases (RAW weights -- emitted before the a-fold) ----
    beff0 = []
    for oh in range(2):
        bp = ot.tile([128, 512], f32, tag="warm", name="bp")
        for t in range(2):
            nc.tensor.matmul(bp[:, 0:1], wb[:, oh, t],
                             bfold_bf[t][:], start=(t == 0), stop=(t == 1))
        bs = pb.tile([128, 1], f32, tag=f"beff0_{oh}", name=f"beff0_{oh}")
        nc.scalar.activation(bs[:], bp[:, 0:1], AF.Identity,
                             bias=cvec[oh][:, 2:3])
        beff0.append(bs)
    b3eff = []
    for oh in range(2):
        bp = ot.tile([128, 512], f32, tag="warm", name="bp3")
        for t in range(2):
            nc.tensor.matmul(bp[:, 0:1], w23t[:, oh, t],
                             bfold_bf[t][:], start=(t == 0), stop=(t == 1))
        bs = pb.tile([128, 1], f32, tag=f"b3eff{oh}", name=f"b3eff{oh}")
        nc.scalar.activation(bs[:], bp[:, 0:1], AF.Identity,
                             bias=cvec[oh][:, 3:4])
        b3eff.append(bs)

    # xqb = x-shard + b3eff (f32); the final stt adds proj*2^-19 onto it.
    # xq is the last input DMA issued -- its transfer queues behind all the
    # early-needed inputs on the serial DMA fabric but lands well before the
    # first finish phase needs xqb.
    xq = pb.tile([128, 2, NSH], f32, tag="xq", name="xq")
    nc.sync.dma_start(xq[:].rearrange("p a b -> p (a b)"), d["xq"][:])
    xqb = pb.tile([128, 2, NSH], f32, tag="xqb", name="xqb")

    def emit_xqb(t):
        # on gpsimd: its queue is idle, so blocking on the late xq DMA is
        # free (on DVE the scheduler head-of-line-blocked the queue)
        nc.gpsimd.tensor_scalar(xqb[:, t], xq[:, t], b3eff[t][:], None,
                                op0=ALU.add)

    # ---- fold a into the fp8 weights directly (one DVE op each) ----
    w018 = pb.tile([128, 2, 2, 128], fp8, tag="w018", name="w018")
    w238 = pb.tile([128, 2, 2, 128], fp8, tag="w238", name="w238")
    for t in range(2):
        nc.vector.tensor_scalar_mul(w018[:, :, t], wb[:, :, t], a_t[t][:])
        nc.vector.tensor_scalar(w238[:, :, t], w23t[:, :, t], a_t[t][:], WS,
                                op0=ALU.mult, op1=ALU.mult)

    if PHASE <= 1:
        _early_out([xq[:, 0], xq[:, 1]])
        ctx.close()
        return

    # ---- q = w0a @ xq + beff0 : fp8 [128, 2(kt=oh), NSH] ----
    # q and g run through the 4-slot ot pool so the 2 big sp slots stay free
    # for the attention pipeline (the sp ring was serializing q -> g -> S).
    q2 = pb.tile([128, 2, NSH], fp8, tag="q2", name="q2")
    for oh in range(2):
        for ch in range(2):
            qp = ot.tile([128, 512], f32, tag="warm", name=f"qp{oh}{ch}")
            nc.tensor.matmul(
                qp[:],
                w018[:, oh],
                xs8[:, ch * 4:(ch + 1) * 4].rearrange(
                    "p mt kt j -> p kt mt j"),
                start=True, stop=True, perf_mode=DR,
            )
            dst = q2[:, oh, ch * 512:(ch + 1) * 512]
            if (oh + ch) % 2 == 0:
                nc.scalar.activation(dst, qp[:], AF.Identity,
                                     bias=beff0[oh][:])
            else:
                nc.vector.tensor_scalar(dst, qp[:], beff0[oh][:], None,
                                        op0=ALU.add)

    # ---- g = a * (w1^T q) : fp8 [128, 2(kt=c-slice), NSH] ----
    g8 = pb.tile([128, 2, NSH], fp8, tag="g8", name="g8")
    for h in range(2):
        for cs in range(2):
            gp = ot.tile([128, 512], f32, tag="warm", name=f"gp{cs}{h}")
            nc.tensor.matmul(
                gp[:],
                w1p8[:, cs],
                q2[:, :, h * 512:(h + 1) * 512],
                start=True, stop=True, perf_mode=DR,
            )
            dst = g8[:, cs, h * 512:(h + 1) * 512]
            if (cs + h) % 2 == 0:
                nc.scalar.activation(dst, gp[:], AF.Copy, scale=a_t[cs][:])
            else:
                nc.vector.tensor_scalar_mul(dst, gp[:], a_t[cs][:])

    if PHASE == 2:
        _early_out([xq[:, 0], xq[:, 1]])
        ctx.close()
        return

    # ---- attention ----
    yts = [pb.tile([128, NSH], f32, tag=f"yts{t}", name=f"yts{t}")
           for t in range(2)]
    # attnx8[p, t(kt for proj), half, n] -- normalized attention-averaged x
    attnx8 = pb.tile([128, 2, 2, 512], fp8, tag="attnx8", name="attnx8")

    def emit_s(half, pr):
        # S^T pair: S[m, n] = sum_c x[c, m] g[c, n]
        st = sp.tile([128, 1024], f32, tag="sp", name="st")
        for h in range(2):
            nc.tensor.matmul(
                st[:, h * 512:(h + 1) * 512],
                xs8[:, 2 * pr + h],
                g8[:, :, half * 512:(half + 1) * 512],
                start=True, stop=True, perf_mode=DR,
            )
        return st

    def emit_exp(half, pr, st):
        # exp -> fp8e5 E chunks, permuted out to [p, ns, kt(2 mt), j].
        # Separate tiles per engine: co-writing one tile through permuted
        # views serializes ACT->DVE in the scheduler.
        stv = st[:].rearrange("p (kt ns j) -> p kt ns j", kt=2, ns=4)
        a = EXP_SPLIT[half * 16 + pr]
        e1 = pw.tile([128, 3, 2, 128], u8, tag="e1", name="e1", bufs=EBUFS)
        e2 = pw.tile([128, 3, 2, 128], u8, tag="e2", name="e2", bufs=EBUFS)
        ev1 = e1[:].rearrange("p ns kt j -> p kt ns j")
        ev2 = e2[:].rearrange("p ns kt j -> p kt ns j")
        nc.scalar.activation(ev1.bitcast(fp8e5)[:, :, 0:a, :],
                             stv[:, :, 0:a, :], AF.Exp, scale=SCALE)
        nc.vector.tensor_scalar(ev2[:, :, 0:4 - a, :], stv[:, :, a:4, :],
                                SCALE * 4.0 * LOG2E, 60.0,
                                op0=ALU.mult, op1=ALU.add)
        return (a, e1, e2)

    def emit_avx(half, pr, e, ots):
        # AVx accumulation + denominator columns (in ots[0] cols 256:260)
        a, e1, e2 = e
        xr = xTw8[:, pr].rearrange("p t kt j -> p kt t j")
        els = [(e1 if ns < a else e2)[:, ns if ns < a else ns - a
                                      ].bitcast(fp8e5) for ns in range(4)]
        for ns in range(4):
            nc.tensor.matmul(
                ots[ns][:, 0:256], els[ns], xr,
                start=(pr == 0), stop=(pr == 15), perf_mode=DR,
            )
        for ns in range(4):
            nc.tensor.matmul(
                ots[0][:, 256 + ns:257 + ns], els[ns], ones5[:],
                start=(pr == 0), stop=(pr == 15), perf_mode=DR,
            )


    def finish_steps(half, ots, use_sp=False):
        # generator of emission steps; each step is interleaved between the
        # next half's pairs so no engine queue sees a serial finish block.
        rec4 = ptiny.tile([128, 4], f32, tag="rec4", name="rec4")
        nc.vector.reciprocal(rec4[:], ots[0][:, 256:260])
        trps = [None, None]
        ons = []

        def emit_on(ns):
            on = pw.tile([128, 256], f32, tag="on", name="on", bufs=5)
            if ON_ACT[half * 4 + ns]:
                nc.scalar.activation(on[:], ots[ns][:, 0:256], AF.Identity,
                                     scale=rec4[:, ns:ns + 1])
            else:
                nc.vector.tensor_scalar_mul(on[:], ots[ns][:, 0:256],
                                            rec4[:, ns:ns + 1])
            ons.append(on)
            if ns == 0:
                trps[0] = ot.tile([128, 512], f32, tag="warm",
                                  name=f"trp{half}_0")[:]
                trps[1] = ot.tile([128, 512], f32, tag="warm",
                                  name=f"trp{half}_1")[:]
            for t in range(2):
                nc.tensor.transpose(trps[t][:, ns * 128:(ns + 1) * 128],
                                    ons[ns][:, t * 128:(t + 1) * 128], ident)

        def emit_drain(t):
            dst = attnx8[:, t, half]
            if TRP_ACT[half * 2 + t]:
                nc.scalar.activation(dst, trps[t], AF.Copy)
            else:
                nc.vector.tensor_copy(dst, trps[t])

        pjs = [None, None]

        def emit_preload(oh):
            # residual preload: exact fp32 identity matmul of xqb*WS into the
            # projection psum; the DR projection then accumulates on top.
            pj = ot.tile([128, 512], f32, tag="warm",
                         name=f"pj{half}_{oh}")[:]
            pjs[oh] = pj
            nc.tensor.matmul(pj, identw,
                             xqb[:, oh, half * 512:(half + 1) * 512],
                             start=True, stop=False, skip_group_check=True)

        def emit_proj(oh):
            pj = pjs[oh]
            nc.tensor.matmul(pj, w238[:, oh], attnx8[:, :, half],
                             start=False, stop=True, perf_mode=DR,
                             skip_group_check=True)
            sl = yts[oh][:, half * 512:(half + 1) * 512]
            if (half + oh) % 2 == 0:
                nc.scalar.activation(sl, pj, AF.Copy, scale=1.0 / WS)
            else:
                nc.vector.tensor_scalar_mul(sl, pj, 1.0 / WS)
            dq = (nc.sync, nc.scalar)[(half + oh) % 2]
            dq.dma_start(
                d["y"][:, oh * NSH + half * 512:oh * NSH + (half + 1) * 512],
                sl,
            )

        def emit_proj_stt(oh):
            # tail half: DVE is free -> plain projection + stt keeps the
            # in-order PE stream short.
            pj = ot.tile([128, 512], f32, tag="warm",
                         name=f"pj{half}_{oh}")[:]
            nc.tensor.matmul(pj, w238[:, oh], attnx8[:, :, half],
                             start=True, stop=True, perf_mode=DR)
            sl = yts[oh][:, half * 512:(half + 1) * 512]
            nc.vector.scalar_tensor_tensor(
                sl, pj, 1.0 / WS, xqb[:, oh, half * 512:(half + 1) * 512],
                op0=ALU.mult, op1=ALU.add,
            )
            dq = (nc.sync, nc.scalar)[(half + oh) % 2]
            dq.dma_start(
                d["y"][:, oh * NSH + half * 512:oh * NSH + (half + 1) * 512],
                sl,
            )

        yield lambda: emit_on(0)
        yield lambda: emit_on(1)
        yield lambda: emit_on(2)
        yield lambda: emit_on(3)
        yield lambda: emit_drain(0)
        yield lambda: emit_drain(1)
        if not use_sp:
            yield lambda: emit_preload(0)
            yield lambda: emit_preload(1)
            yield lambda: emit_proj(0)
            yield lambda: emit_proj(1)
        else:
            yield lambda: emit_proj_stt(0)
            yield lambda: emit_proj_stt(1)

    pending = iter([lambda: emit_xqb(0), lambda: emit_xqb(1)])
    for half in range(2):
        ots = [ot.tile([128, 512], f32, tag="warm", name=f"ot{half}_{ns}")
               for ns in range(4)]
        # PE-stream order per pair p: exp(p), S(p+2), AVx(p) -- the next-next
        # S only needs exp(p)'s slot, so it must not sit behind AVx(p) in the
        # in-order PE queue (that chained AVx latency into every exp cycle).
        sts = {0: emit_s(half, 0), 1: emit_s(half, 1)}
        for pr in range(16):
            e_cur = emit_exp(half, pr, sts.pop(pr))
            if pr + 2 < 16:
                sts[pr + 2] = emit_s(half, pr + 2)
            emit_avx(half, pr, e_cur, ots)
            if pending is not None:
                nxt = next(pending, None)
                if nxt is None:
                    pending = None
                else:
                    nxt()
        pending = finish_steps(half, ots, use_sp=(half == 1))
    for step in pending:
        step()

    ctx.close()


_CACHE = {}


def _get_program():
    if "nc" in _CACHE:
        return _CACHE["nc"], _CACHE["dram"]
    nc = bacc.Bacc("TRN2", target_bir_lowering=False, debug=False,
                   enable_asserts=False, num_devices=NCORES)
    d = {}
    d["xs8"] = nc.dram_tensor("xs8", [128, MT * 256], fp8,
                              kind="ExternalInput").ap()
    d["xTw8"] = nc.dram_tensor("xTw8", [128, 16 * 512], fp8,
                               kind="ExternalInput").ap()
    d["xq"] = nc.dram_tensor("xq", [128, 2 * NSH], f32, kind="ExternalInput").ap()
    d["wb"] = nc.dram_tensor("wb", [128, 4 * 128], bf16, kind="ExternalInput").ap()
    d["w1p8"] = nc.dram_tensor("w1p8", [128, 4 * 128], fp8,
                               kind="ExternalInput").ap()
    d["w23t"] = nc.dram_tensor("w23t", [128, 4 * 128], bf16,
                               kind="ExternalInput").ap()
    d["cpack"] = nc.dram_tensor("cpack", [128, CPW], f32,
                                kind="ExternalInput").ap()
    d["y"] = nc.dram_tensor("y", [128, 2 * NSH], f32, kind="ExternalOutput").ap()

    with tile.TileContext(nc) as tc:
        _build_body(nc, tc, d)
    nc.compile()
    _CACHE["nc"] = nc
    _CACHE["dram"] = d
    return nc, d


def make_in_maps(x, gamma, beta, w0, b0, w1, b1, w2, b2, w3, b3):
    """Host-side sharding/layout prep: returns list of 8 per-core inputs."""
    e4 = ml_dtypes.float8_e4m3
    xb = np.ascontiguousarray(np.asarray(x, np.float32).reshape(B, C, N))

    cpack = np.zeros((128, CPW), np.float32)
    gamma = np.asarray(gamma, np.float32)
    beta = np.asarray(beta, np.float32)
    b0 = np.asarray(b0, np.float32)
    bout = (np.asarray(w3, np.float32) @ np.asarray(b2, np.float32)
            + np.asarray(b3, np.float32))
    for t, off in ((0, CV0), (1, CV1)):
        sl = slice(t * 128, (t + 1) * 128)
        cpack[:, off + 0] = gamma[sl]
        cpack[:, off + 1] = beta[sl]
        cpack[:, off + 2] = b0[sl]
        cpack[:, off + 3] = bout[sl]
        cpack[:, off + 4] = -gamma[sl]
    for t, off in ((0, GMA), (1, GMB)):
        ch = t * 128 + np.arange(128)
        cpack[np.arange(128), off + ch // CPG] = 1.0 / NPG
    for t, off in ((0, GTA), (1, GTB)):
        ch = t * 128 + np.arange(128)
        cpack[ch // CPG, off + np.arange(128)] = gamma[ch]
    cpack[:, IDT:IDT + 128] = np.eye(128, dtype=np.float32)
    cpack[:, IDW:IDW + 128] = np.eye(128, dtype=np.float32) * WS
    cpack[:, EPC] = EPS

    # wb[p, oh, kt, j] = w0^T[kt*128+p, oh*128+j]
    w0t = np.asarray(w0, np.float32).T.reshape(2, 128, 2, 128)  # [kt, p, oh, j]
    wb = w0t.transpose(1, 2, 0, 3).reshape(128, -1).astype(ml_dtypes.bfloat16)
    # w1p8[p, cs, kto, j] = w1[kto*128+p, cs*128+j]
    w1a = np.asarray(w1, np.float32).reshape(2, 128, 2, 128)    # [kto, p, cs, j]
    w1p8 = w1a.transpose(1, 2, 0, 3).reshape(128, -1).astype(e4)
    w23 = (np.asarray(w3, np.float32) @ np.asarray(w2, np.float32)).T
    w23t = w23.reshape(2, 128, 2, 128).transpose(1, 2, 0, 3)
    w23t = w23t.reshape(128, -1).astype(ml_dtypes.bfloat16)

    in_maps = []
    for core in range(NCORES):
        b, j = divmod(core, 4)
        # rotate the token blocks so this core's query shard is mt 0..7;
        # attention sums over m, so any consistent xs8/xTw8 order works
        rot = np.roll(np.arange(MT), -j * 8)
        xc = xb[b]
        xs8 = xc.reshape(2, 128, MT, 128).transpose(1, 2, 0, 3)[:, rot]
        xT = xc.reshape(2, 128, 16, 2, 128)  # [t, jj, pr, kt, p]
        xT = xT.transpose(4, 2, 3, 0, 1).reshape(128, MT, 2, 128)
        xT = xT[:, rot].reshape(128, 16, 2, 2, 128).transpose(0, 1, 3, 2, 4)
        xqc = xc[:, j * NSH:(j + 1) * NSH]
        xq = xqc.reshape(2, 128, NSH).transpose(1, 0, 2).reshape(128, -1)
        m = {
            "xs8": xs8.reshape(128, -1).astype(e4),
            "xTw8": np.ascontiguousarray(xT).reshape(128, -1).astype(e4),
            "xq": np.ascontiguousarray(xq),
            "wb": wb, "w1p8": w1p8, "w23t": w23t, "cpack": cpack,
        }
        in_maps.append(m)
    return in_maps


def assemble_output(results):
    out = np.zeros((B, C, N), np.float32)
    for core in range(NCORES):
        b, j = divmod(core, 4)
        y = results[core]["y"].reshape(128, 2, NSH).transpose(1, 0, 2)
        out[b][:, j * NSH:(j + 1) * NSH] = y.reshape(C, NSH)
    return out.reshape(B, C, 16, 16, 16)


def kernel(x, gamma, beta, w0, b0, w1, b1, w2, b2, w3, b3):
    nc, _ = _get_program()
    in_maps = make_in_maps(x, gamma, beta, w0, b0, w1, b1, w2, b2, w3, b3)
    res = bass_utils.run_bass_kernel_spmd(nc, in_maps, core_ids=list(range(NCORES)))
    return assemble_output(res.results)


# revision 35
# speedup vs baseline: 1.1248x; 1.0095x over previous
"""Trainium2 Bass/Tile kernel for AttnBlock:
GroupNorm(32) -> 1x1 conv q,k,v -> softmax attention over N=4096 tokens
-> 1x1 conv proj -> residual.

Sharding: 8 cores = 2 (batch) x 4 (query-token shards of N).  Each core gets
the full x of its batch plus its n-shard slice, and produces the [C, N/4]
output shard.  No collectives.

Architecture (v4):
- All heavy matmuls are fp8 MatmulPerfMode.DoubleRow: the full K=256
  contraction in one instruction at 0.5 cycles/output-column.  DR stationary
  operands need their 256 weight elements contiguous per partition; every
  lhsT is laid out [.., kt(2), 128].
- GroupNorm stats via a PE Gram-matrix over the m-major fp8 x copy
  (diag -> sum x^2, ones-matmul -> sum x), diag extracted by one DVE
  scalar_tensor_tensor+accum per c-tile.
- No k tensor: S^T = x^T g with g = a*(w1^T q) [C, NSH] -- the PSUM->SBUF
  drain is the n-shard-sized g (2K lanes) instead of the m-sized k (8K).
  The k bias is dropped exactly (softmax shift invariance); q keeps its
  effective bias.
- No v tensor: attention accumulates over x itself:
  AVx[n, c] = sum_m E[m, n] x[c, m] (moving operand = resident xTw8),
  plus denominator columns from a tiny ones matmul per ns.  After
  normalize + transpose, ONE DoubleRow projection by w238 = a*(w3 w2)^T
  (host-folded w3@w2, scaled 2^19 for fp8) produces the output; the scale
  is undone in the final scalar_tensor_tensor against xqb = x + b3eff.
- Softmax over 2-bank [128,1024] S^T psum tiles; exp ns-subtiles split
  between ACT (true Exp -> fp8e5) and DVE (Schraudolph bits =
  round(logit*4*log2e + 60) as uint8 == fp8e5m2; e5m2 because logits span
  +-8).  Output APs are permuted so E tiles come out [ns, kt, j] -- the
  DR lhsT layout for AVx.
"""

import ml_dtypes
import numpy as np

import concourse.bacc as bacc
import concourse.mybir as mybir
import concourse.tile as tile
from concourse import bass_utils

f32 = mybir.dt.float32
bf16 = mybir.dt.bfloat16
fp8 = mybir.dt.float8e4
fp8e5 = mybir.dt.float8e5
u8 = mybir.dt.uint8
AF = mybir.ActivationFunctionType
ALU = mybir.AluOpType
DR = mybir.MatmulPerfMode.DoubleRow

B = 2
C = 256
N = 4096          # 16**3 tokens
NSH = N // 4      # 1024 tokens per core
G = 32
CPG = C // G      # channels per group
NPG = CPG * N     # elements per group
EPS = 1e-6
SCALE = C ** -0.5          # 1/16
LOG2E = float(1.0 / np.log(2.0))
WS = 524288.0              # 2^19 fp8-range scale on w23; undone in the stt
MT = N // 128              # 32 m-tiles

NCORES = 8

# cpack column layout
CV0, CV1 = 0, 8            # cvec slice0/1: [gamma, beta, b0, bout, -gamma]
GMA, GMB = 16, 48          # gmask per slice [128, 32] (1/NPG folded)
GTA, GTB = 80, 208         # gmaskT per slice [32, 128] on partitions 0:32
MZL = 336                  # zeros[128] | ident[128] | zeros[128]
IDT = 464
EPC = 720                  # eps column
IDW = 728                  # identity * WS (residual preload)
CPW = 856

# engine splits (True -> ACT, False -> DVE)
EXP_SPLIT = [2] * 24 + [3] * 8  # ns-subtiles on ACT per (half*16+pair)
GEP_ACT = [True, False]    # g drain per c-slice
ON_ACT = [True, False, True, False, False, True, False, True]
TRP_ACT = [True, False, True, False]  # attnx drain per (half*2 + t)

N_WARMUP = 42
EBUFS = 8
PHASE = 4


def _build_body(nc, tc, d):
    from contextlib import ExitStack

    ctx = ExitStack()
    pc = ctx.enter_context(tc.tile_pool(name="const", bufs=1))
    pb = ctx.enter_context(tc.tile_pool(name="big", bufs=1))
    pw = ctx.enter_context(tc.tile_pool(name="work", bufs=3))
    ptiny = ctx.enter_context(tc.tile_pool(name="tiny", bufs=2))
    # PSUM: sp = 2 x [128,1024] (2 banks each), ot = 4 x [128,512] (1 bank)
    sp = ctx.enter_context(tc.tile_pool(name="sp", bufs=2, space="PSUM"))
    ot = ctx.enter_context(tc.tile_pool(name="pot", bufs=4, space="PSUM"))

    # ---- tiny consts ----
    zcol = pc.tile([128, 1], f32, tag="zcol", name="zcol")
    nc.vector.memset(zcol[:], 0.0)
    nc.const_aps.aps[(f32, 0.0)] = zcol[:]
    ones4 = pc.tile([128, 2, 1], fp8, tag="ones4", name="ones4")
    nc.vector.memset(ones4[:], 1.0)
    ones5 = pc.tile([128, 2, 1], fp8e5, tag="ones5", name="ones5")
    nc.vector.memset(ones5[:], 1.0)

    # ---- PE warmup: dep-free matmuls bridge the DMA head + pstate ramp
    wdum = pc.tile([128, 128], bf16, tag="wdum", name="wdum")
    nc.vector.memset(wdum[:], 1.0)
    wslot = ot.tile([128, 512], f32, tag="warm", name="warm")
    for i in range(N_WARMUP):
        nc.tensor.matmul(wslot[:, 0:128], wdum[:], wdum[:],
                         start=True, stop=True)

    # ---- input DMAs: the DMA fabric is serial -- order by need.
    # xTw8[p, pr, t, kt, j] = x[t*128+j, (2*pr+kt)*128+p], in quarters
    xTw8 = pb.tile([128, 16, 2, 2, 128], fp8, tag="xTw8", name="xTw8")
    xTw8f = xTw8[:].rearrange("p a b c e -> p (a b c e)")
    qs = [nc.sync, nc.scalar]
    cpack = pc.tile([128, CPW], f32, tag="cpack", name="cpack")
    for qr in range(4):
        qs[qr % 2].dma_start(xTw8f[:, qr * 2048:(qr + 1) * 2048],
                             d["xTw8"][:, qr * 2048:(qr + 1) * 2048])
        if qr == 1:
            nc.sync.dma_start(cpack[:], d["cpack"][:])
    # wb[p, oh, kt, j] = w0^T[kt*128+p, oh*128+j]
    wb = pb.tile([128, 2, 2, 128], bf16, tag="wb", name="wb")
    nc.sync.dma_start(wb[:].rearrange("p a b c -> p (a b c)"), d["wb"][:])
    # w1p8[p, cs, kto, j] = w1[kto*128+p, cs*128+j]  (plain w1, fp8)
    w1p8 = pb.tile([128, 2, 2, 128], fp8, tag="w1p8", name="w1p8")
    nc.scalar.dma_start(w1p8[:].rearrange("p a b c -> p (a b c)"), d["w1p8"][:])
    # w23t[p, oh, kt, j] = (w3 w2)^T[kt*128+p, oh*128+j]
    w23t = pb.tile([128, 2, 2, 128], bf16, tag="w23t", name="w23t")
    nc.sync.dma_start(w23t[:].rearrange("p a b c -> p (a b c)"), d["w23t"][:])
    # xs8[p, mt, kt, j] = x[kt*128+p, mt*128+j], halves
    xs8 = pb.tile([128, MT, 2, 128], fp8, tag="xs8", name="xs8")
    xs8f = xs8[:].rearrange("p a b c -> p (a b c)")
    nc.scalar.dma_start(xs8f[:, 0:4096], d["xs8"][:, 0:4096])
    nc.sync.dma_start(xs8f[:, 4096:8192], d["xs8"][:, 4096:8192])
    # xq (f32 residual) is emitted LAST -- only needed by the final stt

    cvec = [cpack[:, CV0:CV0 + 8], cpack[:, CV1:CV1 + 8]]
    gm = [cpack[:, GMA:GMA + 32], cpack[:, GMB:GMB + 32]]
    gmt = [cpack[0:32, GTA:GTA + 128], cpack[0:32, GTB:GTB + 128]]
    ident = cpack[:, IDT:IDT + 128]
    dmask = [cpack[:, IDT:IDT + 256], cpack[:, MZL:MZL + 256]]
    identw = cpack[:, IDW:IDW + 128]
    epscol = cpack[0:32, EPC:EPC + 1]

    # ---- GroupNorm stats via PE Gram over xTw8 ----
    # gtile: [t0 gram 0:256 | t0 sum-x 256 | pad | t1 gram 512:768 | t1 sum-x]
    gtile = sp.tile([128, 1024], f32, tag="sp", name="gram")
    goff = [0, 512]
    for pr in range(16):
        for t in range(2):
            lhs = xTw8[:, pr, t]
            nc.tensor.matmul(
                gtile[:, goff[t]:goff[t] + 256],
                lhs,
                xTw8[:, pr].rearrange("p t kt j -> p kt t j"),
                start=(pr == 0), stop=(pr == 15), perf_mode=DR,
            )
            nc.tensor.matmul(
                gtile[:, goff[t] + 256:goff[t] + 257],
                lhs, ones4[:],
                start=(pr == 0), stop=(pr == 15), perf_mode=DR,
            )
    # pt[t]: col0 = sum x, col1 = sum x^2 (diag extract)
    pt = [ptiny.tile([128, 2], f32, tag=f"pt{t}", name=f"pt{t}") for t in range(2)]
    trash = pw.tile([128, 256], f32, tag="trash", name="trash", bufs=2)
    for t in range(2):
        nc.vector.tensor_copy(pt[t][:, 0:1], gtile[:, goff[t] + 256:goff[t] + 257])
        nc.vector.scalar_tensor_tensor(
            trash[:], gtile[:, goff[t]:goff[t] + 256], 1.0, dmask[t],
            op0=ALU.mult, op1=ALU.mult, accum_out=pt[t][:, 1:2],
        )

    # group stats -> mu/rstd -> per-channel a, bfold
    s32 = ot.tile([128, 512], f32, tag="warm", name="s32")
    for t in range(2):
        nc.tensor.matmul(s32[0:32, 0:2], gm[t], pt[t][:],
                         start=(t == 0), stop=(t == 1))
    sg = ptiny.tile([32, 2], f32, tag="sg", name="sg")
    nc.vector.tensor_copy(sg[:], s32[0:32, 0:2])
    mr = ptiny.tile([32, 2], f32, tag="mr", name="mr")
    musq = ptiny.tile([32, 1], f32, tag="musq", name="musq")
    nc.vector.tensor_mul(musq[:], sg[:, 0:1], sg[:, 0:1])
    var = ptiny.tile([32, 1], f32, tag="var", name="var")
    nc.vector.tensor_sub(var[:], sg[:, 1:2], musq[:])
    std = ptiny.tile([32, 1], f32, tag="std", name="std")
    nc.scalar.activation(std[:], var[:], AF.Sqrt, bias=epscol)
    # dummy Exp preloads the exp table during the DMA head
    edum = ptiny.tile([128, 1], u8, tag="edum", name="edum")
    nc.scalar.activation(edum[:].bitcast(fp8e5), zcol[:], AF.Exp)
    nc.vector.reciprocal(mr[:, 1:2], std[:])
    nc.vector.tensor_mul(mr[:, 0:1], sg[:, 0:1], mr[:, 1:2])

    # gmaskT carries gamma (host-folded): bc = [mu*rstd*gamma, rstd*gamma=a]
    a_t, bfold_bf = [], []
    bc = ot.tile([128, 512], f32, tag="warm", name="bc")
    for t in range(2):
        nc.tensor.matmul(bc[:, 2 * t:2 * t + 2], gmt[t], mr[:],
                         start=True, stop=True)
    for t in range(2):
        a = pb.tile([128, 1], f32, tag=f"a{t}", name=f"a{t}")
        nc.vector.tensor_copy(a[:], bc[:, 2 * t + 1:2 * t + 2])
        bb = pb.tile([128, 1], bf16, tag=f"bfb{t}", name=f"bfb{t}")
        nc.vector.tensor_scalar(bb[:], bc[:, 2 * t:2 * t + 1], -1.0,
                                cvec[t][:, 1:2], op0=ALU.mult, op1=ALU.add)
        a_t.append(a)
        bfold_bf.append(bb)

    def _early_out(srcs):
        for oh in range(2):
            for ch in range(2):
                yt = pw.tile([128, 512], f32, tag="yt", name="yt")
                nc.vector.tensor_copy(
                    yt[:], srcs[oh][:, ch * 512:(ch + 1) * 512])
                nc.sync.dma_start(
                    d["y"][:, oh * NSH + ch * 512:oh * NSH + (ch + 1) * 512],
                    yt[:])

    # ---- effective biases (RAW weights -- emitted before the a-fold) ----
    beff0 = []
    for oh in range(2):
        bp = ot.tile([128, 512], f32, tag="warm", name="bp")
        for t in range(2):
            nc.tensor.matmul(bp[:, 0:1], wb[:, oh, t],
                             bfold_bf[t][:], start=(t == 0), stop=(t == 1))
        bs = pb.tile([128, 1], f32, tag=f"beff0_{oh}", name=f"beff0_{oh}")
        nc.scalar.activation(bs[:], bp[:, 0:1], AF.Identity,
                             bias=cvec[oh][:, 2:3])
        beff0.append(bs)
    b3eff = []
    for oh in range(2):
        bp = ot.tile([128, 512], f32, tag="warm", name="bp3")
        for t in range(2):
            nc.tensor.matmul(bp[:, 0:1], w23t[:, oh, t],
                             bfold_bf[t][:], start=(t == 0), stop=(t == 1))
        bs = pb.tile([128, 1], f32, tag=f"b3eff{oh}", name=f"b3eff{oh}")
        nc.scalar.activation(bs[:], bp[:, 0:1], AF.Identity,
                             bias=cvec[oh][:, 3:4])
        b3eff.append(bs)

    # xqb = x-shard + b3eff (f32); the final stt adds proj*2^-19 onto it.
    # xq is the last input DMA issued -- its transfer queues behind all the
    # early-needed inputs on the serial DMA fabric but lands well before the
    # first finish phase needs xqb.
    xq = pb.tile([128, 2, NSH], f32, tag="xq", name="xq")
    nc.sync.dma_start(xq[:].rearrange("p a b -> p (a b)"), d["xq"][:])
    xqb = pb.tile([128, 2, NSH], f32, tag="xqb", name="xqb")

    def emit_xqb(t):
        # on gpsimd: its queue is idle, so blocking on the late xq DMA is
        # free (on DVE the scheduler head-of-line-blocked the queue)
        nc.gpsimd.tensor_scalar(xqb[:, t], xq[:, t], b3eff[t][:], None,
                                op0=ALU.add)

    # ---- fold a into the fp8 weights directly (one DVE op each) ----
    w018 = pb.tile([128, 2, 2, 128], fp8, tag="w018", name="w018")
    w238 = pb.tile([128, 2, 2, 128], fp8, tag="w238", name="w238")
    for t in range(2):
        nc.vector.tensor_scalar_mul(w018[:, :, t], wb[:, :, t], a_t[t][:])
        nc.vector.tensor_scalar(w238[:, :, t], w23t[:, :, t], a_t[t][:], WS,
                                op0=ALU.mult, op1=ALU.mult)

    if PHASE <= 1:
        _early_out([xq[:, 0], xq[:, 1]])
        ctx.close()
        return

    # ---- q = w0a @ xq + beff0 : fp8 [128, 2(kt=oh), NSH] ----
    # q and g run through the 4-slot ot pool so the 2 big sp slots stay free
    # for the attention pipeline (the sp ring was serializing q -> g -> S).
    q2 = pb.tile([128, 2, NSH], fp8, tag="q2", name="q2")
    for ch in range(2):
        for oh in range(2):
            qp = ot.tile([128, 512], f32, tag="warm", name=f"qp{oh}{ch}")
            nc.tensor.matmul(
                qp[:],
                w018[:, oh],
                xs8[:, ch * 4:(ch + 1) * 4].rearrange(
                    "p mt kt j -> p kt mt j"),
                start=True, stop=True, perf_mode=DR,
            )
            dst = q2[:, oh, ch * 512:(ch + 1) * 512]
            if (oh + ch) % 2 == 0:
                nc.scalar.activation(dst, qp[:], AF.Identity,
                                     bias=beff0[oh][:])
            else:
                nc.vector.tensor_scalar(dst, qp[:], beff0[oh][:], None,
                                        op0=ALU.add)

    # ---- g = a * (w1^T q) : fp8 [128, 2(kt=c-slice), NSH] ----
    g8 = pb.tile([128, 2, NSH], fp8, tag="g8", name="g8")
    for h in range(2):
        for cs in range(2):
            gp = ot.tile([128, 512], f32, tag="warm", name=f"gp{cs}{h}")
            nc.tensor.matmul(
                gp[:],
                w1p8[:, cs],
                q2[:, :, h * 512:(h + 1) * 512],
                start=True, stop=True, perf_mode=DR,
            )
            dst = g8[:, cs, h * 512:(h + 1) * 512]
            if (cs + h) % 2 == 0:
                nc.scalar.activation(dst, gp[:], AF.Copy, scale=a_t[cs][:])
            else:
                nc.vector.tensor_scalar_mul(dst, gp[:], a_t[cs][:])

    if PHASE == 2:
        _early_out([xq[:, 0], xq[:, 1]])
        ctx.close()
        return

    # ---- attention ----
    yts = [pb.tile([128, NSH], f32, tag=f"yts{t}", name=f"yts{t}")
           for t in range(2)]
    # attnx8[p, t(kt for proj), half, n] -- normalized attention-averaged x
    attnx8 = pb.tile([128, 2, 2, 512], fp8, tag="attnx8", name="attnx8")

    def emit_s(half, pr):
        # S^T pair: S[m, n] = sum_c x[c, m] g[c, n]
        st = sp.tile([128, 1024], f32, tag="sp", name="st")
        for h in range(2):
            nc.tensor.matmul(
                st[:, h * 512:(h + 1) * 512],
                xs8[:, 2 * pr + h],
                g8[:, :, half * 512:(half + 1) * 512],
                start=True, stop=True, perf_mode=DR,
            )
        return st

    def emit_exp(half, pr, st):
        # exp -> fp8e5 E chunks, permuted out to [p, ns, kt(2 mt), j].
        # Separate tiles per engine: co-writing one tile through permuted
        # views serializes ACT->DVE in the scheduler.
        stv = st[:].rearrange("p (kt ns j) -> p kt ns j", kt=2, ns=4)
        a = EXP_SPLIT[half * 16 + pr]
        e1 = pw.tile([128, 3, 2, 128], u8, tag="e1", name="e1", bufs=EBUFS)
        e2 = pw.tile([128, 3, 2, 128], u8, tag="e2", name="e2", bufs=EBUFS)
        ev1 = e1[:].rearrange("p ns kt j -> p kt ns j")
        ev2 = e2[:].rearrange("p ns kt j -> p kt ns j")
        nc.scalar.activation(ev1.bitcast(fp8e5)[:, :, 0:a, :],
                             stv[:, :, 0:a, :], AF.Exp, scale=SCALE)
        nc.vector.tensor_scalar(ev2[:, :, 0:4 - a, :], stv[:, :, a:4, :],
                                SCALE * 4.0 * LOG2E, 60.0,
                                op0=ALU.mult, op1=ALU.add)
        return (a, e1, e2)

    def emit_avx(half, pr, e, ots):
        # AVx accumulation + denominator columns (in ots[0] cols 256:260)
        a, e1, e2 = e
        xr = xTw8[:, pr].rearrange("p t kt j -> p kt t j")
        els = [(e1 if ns < a else e2)[:, ns if ns < a else ns - a
                                      ].bitcast(fp8e5) for ns in range(4)]
        for ns in range(4):
            nc.tensor.matmul(
                ots[ns][:, 0:256], els[ns], xr,
                start=(pr == 0), stop=(pr == 15), perf_mode=DR,
            )
        for ns in range(4):
            nc.tensor.matmul(
                ots[0][:, 256 + ns:257 + ns], els[ns], ones5[:],
                start=(pr == 0), stop=(pr == 15), perf_mode=DR,
            )


    def finish_steps(half, ots, use_sp=False):
        # generator of emission steps; each step is interleaved between the
        # next half's pairs so no engine queue sees a serial finish block.
        rec4 = ptiny.tile([128, 4], f32, tag="rec4", name="rec4")
        nc.vector.reciprocal(rec4[:], ots[0][:, 256:260])
        trps = [None, None]
        ons = []

        def emit_on(ns):
            on = pw.tile([128, 256], f32, tag="on", name="on", bufs=5)
            if ON_ACT[half * 4 + ns]:
                nc.scalar.activation(on[:], ots[ns][:, 0:256], AF.Identity,
                                     scale=rec4[:, ns:ns + 1])
            else:
                nc.vector.tensor_scalar_mul(on[:], ots[ns][:, 0:256],
                                            rec4[:, ns:ns + 1])
            ons.append(on)
            if ns == 0:
                trps[0] = ot.tile([128, 512], f32, tag="warm",
                                  name=f"trp{half}_0")[:]
                trps[1] = ot.tile([128, 512], f32, tag="warm",
                                  name=f"trp{half}_1")[:]
            for t in range(2):
                nc.tensor.transpose(trps[t][:, ns * 128:(ns + 1) * 128],
                                    ons[ns][:, t * 128:(t + 1) * 128], ident)

        def emit_drain(t):
            dst = attnx8[:, t, half]
            if TRP_ACT[half * 2 + t]:
                nc.scalar.activation(dst, trps[t], AF.Copy)
            else:
                nc.vector.tensor_copy(dst, trps[t])

        pjs = [None, None]

        def emit_preload(oh):
            # residual preload: exact fp32 identity matmul of xqb*WS into the
            # projection psum; the DR projection then accumulates on top.
            if use_sp:
                if pjs[0] is None:
                    pp = sp.tile([128, 1024], f32, tag="sp", name=f"pj{half}")
                    pjs[0], pjs[1] = pp[:, 0:512], pp[:, 512:1024]
                pj = pjs[oh]
            else:
                pj = ot.tile([128, 512], f32, tag="warm",
                             name=f"pj{half}_{oh}")[:]
                pjs[oh] = pj
            nc.tensor.matmul(pj, identw,
                             xqb[:, oh, half * 512:(half + 1) * 512],
                             start=True, stop=False, skip_group_check=True)

        def emit_proj(oh):
            pj = pjs[oh]
            nc.tensor.matmul(pj, w238[:, oh], attnx8[:, :, half],
                             start=False, stop=True, perf_mode=DR,
                             skip_group_check=True)
            sl = yts[oh][:, half * 512:(half + 1) * 512]
            if (half + oh) % 2 == 0:
                nc.scalar.activation(sl, pj, AF.Copy, scale=1.0 / WS)
            else:
                nc.vector.tensor_scalar_mul(sl, pj, 1.0 / WS)
            dq = (nc.sync, nc.scalar)[(half + oh) % 2]
            dq.dma_start(
                d["y"][:, oh * NSH + half * 512:oh * NSH + (half + 1) * 512],
                sl,
            )

        def emit_proj_stt(oh):
            # tail half: DVE is free -> plain projection + stt keeps the
            # in-order PE stream short.
            pj = ot.tile([128, 512], f32, tag="warm",
                         name=f"pj{half}_{oh}")[:]
            nc.tensor.matmul(pj, w238[:, oh], attnx8[:, :, half],
                             start=True, stop=True, perf_mode=DR)
            sl = yts[oh][:, half * 512:(half + 1) * 512]
            nc.vector.scalar_tensor_tensor(
                sl, pj, 1.0 / WS, xqb[:, oh, half * 512:(half + 1) * 512],
                op0=ALU.mult, op1=ALU.add,
            )
            dq = (nc.sync, nc.scalar)[(half + oh) % 2]
            dq.dma_start(
                d["y"][:, oh * NSH + half * 512:oh * NSH + (half + 1) * 512],
                sl,
            )

        yield lambda: emit_on(0)
        yield lambda: emit_on(1)
        yield lambda: emit_on(2)
        yield lambda: emit_on(3)
        yield lambda: emit_drain(0)
        yield lambda: emit_drain(1)
        if not use_sp:
            yield lambda: emit_preload(0)
            yield lambda: emit_preload(1)
            yield lambda: emit_proj(0)
            yield lambda: emit_proj(1)
        else:
            yield lambda: emit_proj_stt(0)
            yield lambda: emit_proj_stt(1)

    pending = iter([lambda: emit_xqb(0), lambda: emit_xqb(1)])
    for half in range(2):
        ots = [ot.tile([128, 512], f32, tag="warm", name=f"ot{half}_{ns}")
               for ns in range(4)]
        # PE-stream order per pair p: exp(p), S(p+2), AVx(p) -- the next-next
        # S only needs exp(p)'s slot, so it must not sit behind AVx(p) in the
        # in-order PE queue (that chained AVx latency into every exp cycle).
        sts = {0: emit_s(half, 0), 1: emit_s(half, 1)}
        for pr in range(16):
            e_cur = emit_exp(half, pr, sts.pop(pr))
            if pr + 2 < 16:
                sts[pr + 2] = emit_s(half, pr + 2)
            emit_avx(half, pr, e_cur, ots)
            if pending is not None:
                nxt = next(pending, None)
                if nxt is None:
                    pending = None
                else:
                    nxt()
        pending = finish_steps(half, ots, use_sp=(half == 1))
    for step in pending:
        step()

    ctx.close()


_CACHE = {}


def _get_program():
    if "nc" in _CACHE:
        return _CACHE["nc"], _CACHE["dram"]
    nc = bacc.Bacc("TRN2", target_bir_lowering=False, debug=False,
                   enable_asserts=False, num_devices=NCORES)
    d = {}
    d["xs8"] = nc.dram_tensor("xs8", [128, MT * 256], fp8,
                              kind="ExternalInput").ap()
    d["xTw8"] = nc.dram_tensor("xTw8", [128, 16 * 512], fp8,
                               kind="ExternalInput").ap()
    d["xq"] = nc.dram_tensor("xq", [128, 2 * NSH], f32, kind="ExternalInput").ap()
    d["wb"] = nc.dram_tensor("wb", [128, 4 * 128], bf16, kind="ExternalInput").ap()
    d["w1p8"] = nc.dram_tensor("w1p8", [128, 4 * 128], fp8,
                               kind="ExternalInput").ap()
    d["w23t"] = nc.dram_tensor("w23t", [128, 4 * 128], bf16,
                               kind="ExternalInput").ap()
    d["cpack"] = nc.dram_tensor("cpack", [128, CPW], f32,
                                kind="ExternalInput").ap()
    d["y"] = nc.dram_tensor("y", [128, 2 * NSH], f32, kind="ExternalOutput").ap()

    with tile.TileContext(nc) as tc:
        _build_body(nc, tc, d)
    nc.compile()
    _CACHE["nc"] = nc
    _CACHE["dram"] = d
    return nc, d


def make_in_maps(x, gamma, beta, w0, b0, w1, b1, w2, b2, w3, b3):
    """Host-side sharding/layout prep: returns list of 8 per-core inputs."""
    e4 = ml_dtypes.float8_e4m3
    xb = np.ascontiguousarray(np.asarray(x, np.float32).reshape(B, C, N))

    cpack = np.zeros((128, CPW), np.float32)
    gamma = np.asarray(gamma, np.float32)
    beta = np.asarray(beta, np.float32)
    b0 = np.asarray(b0, np.float32)
    bout = (np.asarray(w3, np.float32) @ np.asarray(b2, np.float32)
            + np.asarray(b3, np.float32))
    for t, off in ((0, CV0), (1, CV1)):
        sl = slice(t * 128, (t + 1) * 128)
        cpack[:, off + 0] = gamma[sl]
        cpack[:, off + 1] = beta[sl]
        cpack[:, off + 2] = b0[sl]
        cpack[:, off + 3] = bout[sl]
        cpack[:, off + 4] = -gamma[sl]
    for t, off in ((0, GMA), (1, GMB)):
        ch = t * 128 + np.arange(128)
        cpack[np.arange(128), off + ch // CPG] = 1.0 / NPG
    for t, off in ((0, GTA), (1, GTB)):
        ch = t * 128 + np.arange(128)
        cpack[ch // CPG, off + np.arange(128)] = gamma[ch]
    cpack[:, IDT:IDT + 128] = np.eye(128, dtype=np.float32)
    cpack[:, IDW:IDW + 128] = np.eye(128, dtype=np.float32) * WS
    cpack[:, EPC] = EPS

    # wb[p, oh, kt, j] = w0^T[kt*128+p, oh*128+j]
    w0t = np.asarray(w0, np.float32).T.reshape(2, 128, 2, 128)  # [kt, p, oh, j]
    wb = w0t.transpose(1, 2, 0, 3).reshape(128, -1).astype(ml_dtypes.bfloat16)
    # w1p8[p, cs, kto, j] = w1[kto*128+p, cs*128+j]
    w1a = np.asarray(w1, np.float32).reshape(2, 128, 2, 128)    # [kto, p, cs, j]
    w1p8 = w1a.transpose(1, 2, 0, 3).reshape(128, -1).astype(e4)
    w23 = (np.asarray(w3, np.float32) @ np.asarray(w2, np.float32)).T
    w23t = w23.reshape(2, 128, 2, 128).transpose(1, 2, 0, 3)
    w23t = w23t.reshape(128, -1).astype(ml_dtypes.bfloat16)

    in_maps = []
    for core in range(NCORES):
        b, j = divmod(core, 4)
        # rotate the token blocks so this core's query shard is mt 0..7;
        # attention sums over m, so any consistent xs8/xTw8 order works
        rot = np.roll(np.arange(MT), -j * 8)
        xc = xb[b]
        xs8 = xc.reshape(2, 128, MT, 128).transpose(1, 2, 0, 3)[:, rot]
        xT = xc.reshape(2, 128, 16, 2, 128)  # [t, jj, pr, kt, p]
        xT = xT.transpose(4, 2, 3, 0, 1).reshape(128, MT, 2, 128)
        xT = xT[:, rot].reshape(128, 16, 2, 2, 128).transpose(0, 1, 3, 2, 4)
        xqc = xc[:, j * NSH:(j + 1) * NSH]
        xq = xqc.reshape(2, 128, NSH).transpose(1, 0, 2).reshape(128, -1)
        m = {
            "xs8": xs8.reshape(128, -1).astype(e4),
            "xTw8": np.ascontiguousarray(xT).reshape(128, -1).astype(e4),
            "xq": np.ascontiguousarray(xq),
            "wb": wb, "w1p8": w1p8, "w23t": w23t, "cpack": cpack,
        }
        in_maps.append(m)
    return in_maps


def assemble_output(results):
    out = np.zeros((B, C, N), np.float32)
    for core in range(NCORES):
        b, j = divmod(core, 4)
        y = results[core]["y"].reshape(128, 2, NSH).transpose(1, 0, 2)
        out[b][:, j * NSH:(j + 1) * NSH] = y.reshape(C, NSH)
    return out.reshape(B, C, 16, 16, 16)


def kernel(x, gamma, beta, w0, b0, w1, b1, w2, b2, w3, b3):
    nc, _ = _get_program()
    in_maps = make_in_maps(x, gamma, beta, w0, b0, w1, b1, w2, b2, w3, b3)
    res = bass_utils.run_bass_kernel_spmd(nc, in_maps, core_ids=list(range(NCORES)))
    return assemble_output(res.results)


# revision 36
# speedup vs baseline: 1.1444x; 1.0173x over previous
"""Trainium2 Bass/Tile kernel for AttnBlock:
GroupNorm(32) -> 1x1 conv q,k,v -> softmax attention over N=4096 tokens
-> 1x1 conv proj -> residual.

Sharding: 8 cores = 2 (batch) x 4 (query-token shards of N).  Each core gets
the full x of its batch plus its n-shard slice, and produces the [C, N/4]
output shard.  No collectives.

Architecture (v4):
- All heavy matmuls are fp8 MatmulPerfMode.DoubleRow: the full K=256
  contraction in one instruction at 0.5 cycles/output-column.  DR stationary
  operands need their 256 weight elements contiguous per partition; every
  lhsT is laid out [.., kt(2), 128].
- GroupNorm stats via a PE Gram-matrix over the m-major fp8 x copy
  (diag -> sum x^2, ones-matmul -> sum x), diag extracted by one DVE
  scalar_tensor_tensor+accum per c-tile.
- No k tensor: S^T = x^T g with g = a*(w1^T q) [C, NSH] -- the PSUM->SBUF
  drain is the n-shard-sized g (2K lanes) instead of the m-sized k (8K).
  The k bias is dropped exactly (softmax shift invariance); q keeps its
  effective bias.
- No v tensor: attention accumulates over x itself:
  AVx[n, c] = sum_m E[m, n] x[c, m] (moving operand = resident xTw8),
  plus denominator columns from a tiny ones matmul per ns.  After
  normalize + transpose, ONE DoubleRow projection by w238 = a*(w3 w2)^T
  (host-folded w3@w2, scaled 2^19 for fp8) produces the output; the scale
  is undone in the final scalar_tensor_tensor against xqb = x + b3eff.
- Softmax over 2-bank [128,1024] S^T psum tiles; exp ns-subtiles split
  between ACT (true Exp -> fp8e5) and DVE (Schraudolph bits =
  round(logit*4*log2e + 60) as uint8 == fp8e5m2; e5m2 because logits span
  +-8).  Output APs are permuted so E tiles come out [ns, kt, j] -- the
  DR lhsT layout for AVx.
"""

import ml_dtypes
import numpy as np

import concourse.bacc as bacc
import concourse.mybir as mybir
import concourse.tile as tile
from concourse import bass_utils

f32 = mybir.dt.float32
bf16 = mybir.dt.bfloat16
fp8 = mybir.dt.float8e4
fp8e5 = mybir.dt.float8e5
u8 = mybir.dt.uint8
AF = mybir.ActivationFunctionType
ALU = mybir.AluOpType
DR = mybir.MatmulPerfMode.DoubleRow

B = 2
C = 256
N = 4096          # 16**3 tokens
NSH = N // 4      # 1024 tokens per core
G = 32
CPG = C // G      # channels per group
NPG = CPG * N     # elements per group
EPS = 1e-6
SCALE = C ** -0.5          # 1/16
LOG2E = float(1.0 / np.log(2.0))
WS = 524288.0              # 2^19 fp8-range scale on w23; undone in the stt
MT = N // 128              # 32 m-tiles

NCORES = 8

# cpack column layout
CV0, CV1 = 0, 8            # cvec slice0/1: [gamma, beta, b0, bout, -gamma]
GMA, GMB = 16, 48          # gmask per slice [128, 32] (1/NPG folded)
GTA, GTB = 80, 208         # gmaskT per slice [32, 128] on partitions 0:32
MZL = 336                  # zeros[128] | ident[128] | zeros[128]
IDT = 464
EPC = 720                  # eps column
IDW = 728                  # identity * WS (residual preload)
CPW = 856

# engine splits (True -> ACT, False -> DVE)
EXP_SPLIT = [2] * 24 + [3] * 8  # ns-subtiles on ACT per (half*16+pair)
GEP_ACT = [True, False]    # g drain per c-slice
ON_ACT = [True, False, True, False, False, True, False, True]
TRP_ACT = [True, False, True, False]  # attnx drain per (half*2 + t)

N_WARMUP = 42
EBUFS = 8
PHASE = 4


def _build_body(nc, tc, d):
    from contextlib import ExitStack

    ctx = ExitStack()
    pc = ctx.enter_context(tc.tile_pool(name="const", bufs=1))
    pb = ctx.enter_context(tc.tile_pool(name="big", bufs=1))
    pw = ctx.enter_context(tc.tile_pool(name="work", bufs=3))
    ptiny = ctx.enter_context(tc.tile_pool(name="tiny", bufs=2))
    # PSUM: sp = 2 x [128,1024] (2 banks each), ot = 4 x [128,512] (1 bank)
    sp = ctx.enter_context(tc.tile_pool(name="sp", bufs=2, space="PSUM"))
    ot = ctx.enter_context(tc.tile_pool(name="pot", bufs=4, space="PSUM"))

    # ---- tiny consts ----
    zcol = pc.tile([128, 1], f32, tag="zcol", name="zcol")
    nc.vector.memset(zcol[:], 0.0)
    nc.const_aps.aps[(f32, 0.0)] = zcol[:]
    ones4 = pc.tile([128, 2, 1], fp8, tag="ones4", name="ones4")
    nc.vector.memset(ones4[:], 1.0)
    ones5 = pc.tile([128, 2, 1], fp8e5, tag="ones5", name="ones5")
    nc.vector.memset(ones5[:], 1.0)

    # ---- PE warmup: dep-free matmuls bridge the DMA head + pstate ramp
    wdum = pc.tile([128, 128], bf16, tag="wdum", name="wdum")
    nc.vector.memset(wdum[:], 1.0)
    wslot = ot.tile([128, 512], f32, tag="warm", name="warm")
    for i in range(N_WARMUP):
        nc.tensor.matmul(wslot[:, 0:128], wdum[:], wdum[:],
                         start=True, stop=True)

    # ---- input DMAs: the DMA fabric is serial -- order by need.
    # xTw8[p, pr, t, kt, j] = x[t*128+j, (2*pr+kt)*128+p], in quarters
    xTw8 = pb.tile([128, 16, 2, 2, 128], fp8, tag="xTw8", name="xTw8")
    xTw8f = xTw8[:].rearrange("p a b c e -> p (a b c e)")
    qs = [nc.sync, nc.scalar]
    cpack = pc.tile([128, CPW], f32, tag="cpack", name="cpack")
    for qr in range(4):
        qs[qr % 2].dma_start(xTw8f[:, qr * 2048:(qr + 1) * 2048],
                             d["xTw8"][:, qr * 2048:(qr + 1) * 2048])
        if qr == 1:
            nc.sync.dma_start(cpack[:], d["cpack"][:])
    # wb[p, oh, kt, j] = w0^T[kt*128+p, oh*128+j]
    wb = pb.tile([128, 2, 2, 128], bf16, tag="wb", name="wb")
    nc.sync.dma_start(wb[:].rearrange("p a b c -> p (a b c)"), d["wb"][:])
    # w1p8[p, cs, kto, j] = w1[kto*128+p, cs*128+j]  (plain w1, fp8)
    w1p8 = pb.tile([128, 2, 2, 128], fp8, tag="w1p8", name="w1p8")
    nc.scalar.dma_start(w1p8[:].rearrange("p a b c -> p (a b c)"), d["w1p8"][:])
    # w23t[p, oh, kt, j] = (w3 w2)^T[kt*128+p, oh*128+j]
    w23t = pb.tile([128, 2, 2, 128], bf16, tag="w23t", name="w23t")
    nc.sync.dma_start(w23t[:].rearrange("p a b c -> p (a b c)"), d["w23t"][:])
    # xs8[p, mt, kt, j] = x[kt*128+p, mt*128+j], halves
    xs8 = pb.tile([128, MT, 2, 128], fp8, tag="xs8", name="xs8")
    xs8f = xs8[:].rearrange("p a b c -> p (a b c)")
    nc.scalar.dma_start(xs8f[:, 0:4096], d["xs8"][:, 0:4096])
    nc.sync.dma_start(xs8f[:, 4096:8192], d["xs8"][:, 4096:8192])
    # xq (f32 residual) is emitted LAST -- only needed by the final stt

    cvec = [cpack[:, CV0:CV0 + 8], cpack[:, CV1:CV1 + 8]]
    gm = [cpack[:, GMA:GMA + 32], cpack[:, GMB:GMB + 32]]
    gmt = [cpack[0:32, GTA:GTA + 128], cpack[0:32, GTB:GTB + 128]]
    ident = cpack[:, IDT:IDT + 128]
    dmask = [cpack[:, IDT:IDT + 256], cpack[:, MZL:MZL + 256]]
    identw = cpack[:, IDW:IDW + 128]
    epscol = cpack[0:32, EPC:EPC + 1]

    # ---- GroupNorm stats via PE Gram over xTw8 ----
    # gtile: [t0 gram 0:256 | t0 sum-x 256 | pad | t1 gram 512:768 | t1 sum-x]
    gtile = sp.tile([128, 1024], f32, tag="sp", name="gram")
    goff = [0, 512]
    for pr in range(16):
        for t in range(2):
            lhs = xTw8[:, pr, t]
            nc.tensor.matmul(
                gtile[:, goff[t]:goff[t] + 256],
                lhs,
                xTw8[:, pr].rearrange("p t kt j -> p kt t j"),
                start=(pr == 0), stop=(pr == 15), perf_mode=DR,
            )
            nc.tensor.matmul(
                gtile[:, goff[t] + 256:goff[t] + 257],
                lhs, ones4[:],
                start=(pr == 0), stop=(pr == 15), perf_mode=DR,
            )
    # pt[t]: col0 = sum x, col1 = sum x^2 (diag extract)
    pt = [ptiny.tile([128, 2], f32, tag=f"pt{t}", name=f"pt{t}") for t in range(2)]
    trash = pw.tile([128, 256], f32, tag="trash", name="trash", bufs=2)
    for t in range(2):
        nc.vector.tensor_copy(pt[t][:, 0:1], gtile[:, goff[t] + 256:goff[t] + 257])
        nc.vector.scalar_tensor_tensor(
            trash[:], gtile[:, goff[t]:goff[t] + 256], 1.0, dmask[t],
            op0=ALU.mult, op1=ALU.mult, accum_out=pt[t][:, 1:2],
        )

    # group stats -> mu/rstd -> per-channel a, bfold
    s32 = ot.tile([128, 512], f32, tag="warm", name="s32")
    for t in range(2):
        nc.tensor.matmul(s32[0:32, 0:2], gm[t], pt[t][:],
                         start=(t == 0), stop=(t == 1))
    sg = ptiny.tile([32, 2], f32, tag="sg", name="sg")
    nc.vector.tensor_copy(sg[:], s32[0:32, 0:2])
    mr = ptiny.tile([32, 2], f32, tag="mr", name="mr")
    musq = ptiny.tile([32, 1], f32, tag="musq", name="musq")
    nc.vector.tensor_mul(musq[:], sg[:, 0:1], sg[:, 0:1])
    var = ptiny.tile([32, 1], f32, tag="var", name="var")
    nc.vector.tensor_sub(var[:], sg[:, 1:2], musq[:])
    std = ptiny.tile([32, 1], f32, tag="std", name="std")
    nc.scalar.activation(std[:], var[:], AF.Sqrt, bias=epscol)
    # dummy Exp preloads the exp table right after the Sqrt (reading std
    # pins its queue position; a dep-free op gets rescheduled too late)
    edum = ptiny.tile([32, 1], u8, tag="edum", name="edum")
    nc.scalar.activation(edum[:].bitcast(fp8e5), std[:], AF.Exp)
    nc.vector.reciprocal(mr[:, 1:2], std[:])
    nc.vector.tensor_mul(mr[:, 0:1], sg[:, 0:1], mr[:, 1:2])

    # gmaskT carries gamma (host-folded): bc = [mu*rstd*gamma, rstd*gamma=a]
    a_t, bfold_bf = [], []
    bc = ot.tile([128, 512], f32, tag="warm", name="bc")
    for t in range(2):
        nc.tensor.matmul(bc[:, 2 * t:2 * t + 2], gmt[t], mr[:],
                         start=True, stop=True)
    for t in range(2):
        a = pb.tile([128, 1], f32, tag=f"a{t}", name=f"a{t}")
        nc.vector.tensor_copy(a[:], bc[:, 2 * t + 1:2 * t + 2])
        bb = pb.tile([128, 1], bf16, tag=f"bfb{t}", name=f"bfb{t}")
        nc.vector.tensor_scalar(bb[:], bc[:, 2 * t:2 * t + 1], -1.0,
                                cvec[t][:, 1:2], op0=ALU.mult, op1=ALU.add)
        a_t.append(a)
        bfold_bf.append(bb)

    def _early_out(srcs):
        for oh in range(2):
            for ch in range(2):
                yt = pw.tile([128, 512], f32, tag="yt", name="yt")
                nc.vector.tensor_copy(
                    yt[:], srcs[oh][:, ch * 512:(ch + 1) * 512])
                nc.sync.dma_start(
                    d["y"][:, oh * NSH + ch * 512:oh * NSH + (ch + 1) * 512],
                    yt[:])

    # ---- effective biases (RAW weights -- emitted before the a-fold) ----
    beff0 = []
    for oh in range(2):
        bp = ot.tile([128, 512], f32, tag="warm", name="bp")
        for t in range(2):
            nc.tensor.matmul(bp[:, 0:1], wb[:, oh, t],
                             bfold_bf[t][:], start=(t == 0), stop=(t == 1))
        bs = pb.tile([128, 1], f32, tag=f"beff0_{oh}", name=f"beff0_{oh}")
        nc.scalar.activation(bs[:], bp[:, 0:1], AF.Identity,
                             bias=cvec[oh][:, 2:3])
        beff0.append(bs)
    b3eff = []
    for oh in range(2):
        bp = ot.tile([128, 512], f32, tag="warm", name="bp3")
        for t in range(2):
            nc.tensor.matmul(bp[:, 0:1], w23t[:, oh, t],
                             bfold_bf[t][:], start=(t == 0), stop=(t == 1))
        bs = pb.tile([128, 1], f32, tag=f"b3eff{oh}", name=f"b3eff{oh}")
        nc.scalar.activation(bs[:], bp[:, 0:1], AF.Identity,
                             bias=cvec[oh][:, 3:4])
        b3eff.append(bs)

    # xqb = x-shard + b3eff (f32); the final stt adds proj*2^-19 onto it.
    # xq is the last input DMA issued -- its transfer queues behind all the
    # early-needed inputs on the serial DMA fabric but lands well before the
    # first finish phase needs xqb.
    xq = pb.tile([128, 2, NSH], f32, tag="xq", name="xq")
    nc.sync.dma_start(xq[:].rearrange("p a b -> p (a b)"), d["xq"][:])
    xqb = pb.tile([128, 2, NSH], f32, tag="xqb", name="xqb")

    def emit_xqb(t):
        # on gpsimd: its queue is idle, so blocking on the late xq DMA is
        # free (on DVE the scheduler head-of-line-blocked the queue)
        nc.gpsimd.tensor_scalar(xqb[:, t], xq[:, t], b3eff[t][:], None,
                                op0=ALU.add)

    # ---- fold a into the fp8 weights directly (one DVE op each) ----
    w018 = pb.tile([128, 2, 2, 128], fp8, tag="w018", name="w018")
    w238 = pb.tile([128, 2, 2, 128], fp8, tag="w238", name="w238")
    for t in range(2):
        nc.vector.tensor_scalar_mul(w018[:, :, t], wb[:, :, t], a_t[t][:])
        nc.vector.tensor_scalar(w238[:, :, t], w23t[:, :, t], a_t[t][:], WS,
                                op0=ALU.mult, op1=ALU.mult)

    if PHASE <= 1:
        _early_out([xq[:, 0], xq[:, 1]])
        ctx.close()
        return

    # ---- q = w0a @ xq + beff0 : fp8 [128, 2(kt=oh), NSH] ----
    # q and g run through the 4-slot ot pool so the 2 big sp slots stay free
    # for the attention pipeline (the sp ring was serializing q -> g -> S).
    q2 = pb.tile([128, 2, NSH], fp8, tag="q2", name="q2")
    for ch in range(2):
        for oh in range(2):
            qp = ot.tile([128, 512], f32, tag="warm", name=f"qp{oh}{ch}")
            nc.tensor.matmul(
                qp[:],
                w018[:, oh],
                xs8[:, ch * 4:(ch + 1) * 4].rearrange(
                    "p mt kt j -> p kt mt j"),
                start=True, stop=True, perf_mode=DR,
            )
            dst = q2[:, oh, ch * 512:(ch + 1) * 512]
            if (oh + ch) % 2 == 0:
                nc.scalar.activation(dst, qp[:], AF.Identity,
                                     bias=beff0[oh][:])
            else:
                nc.vector.tensor_scalar(dst, qp[:], beff0[oh][:], None,
                                        op0=ALU.add)

    # ---- g = a * (w1^T q) : fp8 [128, 2(kt=c-slice), NSH] ----
    g8 = pb.tile([128, 2, NSH], fp8, tag="g8", name="g8")
    for h in range(2):
        for cs in range(2):
            gp = ot.tile([128, 512], f32, tag="warm", name=f"gp{cs}{h}")
            nc.tensor.matmul(
                gp[:],
                w1p8[:, cs],
                q2[:, :, h * 512:(h + 1) * 512],
                start=True, stop=True, perf_mode=DR,
            )
            dst = g8[:, cs, h * 512:(h + 1) * 512]
            if (cs + h) % 2 == 0:
                nc.scalar.activation(dst, gp[:], AF.Copy, scale=a_t[cs][:])
            else:
                nc.vector.tensor_scalar_mul(dst, gp[:], a_t[cs][:])

    if PHASE == 2:
        _early_out([xq[:, 0], xq[:, 1]])
        ctx.close()
        return

    # ---- attention ----
    yts = [pb.tile([128, NSH], f32, tag=f"yts{t}", name=f"yts{t}")
           for t in range(2)]
    # attnx8[p, t(kt for proj), half, n] -- normalized attention-averaged x
    attnx8 = pb.tile([128, 2, 2, 512], fp8, tag="attnx8", name="attnx8")

    def emit_s(half, pr):
        # S^T pair: S[m, n] = sum_c x[c, m] g[c, n]
        st = sp.tile([128, 1024], f32, tag="sp", name="st")
        for h in range(2):
            nc.tensor.matmul(
                st[:, h * 512:(h + 1) * 512],
                xs8[:, 2 * pr + h],
                g8[:, :, half * 512:(half + 1) * 512],
                start=True, stop=True, perf_mode=DR,
            )
        return st

    def emit_exp(half, pr, st):
        # exp -> fp8e5 E chunks, permuted out to [p, ns, kt(2 mt), j].
        # Separate tiles per engine: co-writing one tile through permuted
        # views serializes ACT->DVE in the scheduler.
        stv = st[:].rearrange("p (kt ns j) -> p kt ns j", kt=2, ns=4)
        a = EXP_SPLIT[half * 16 + pr]
        e1 = pw.tile([128, 3, 2, 128], u8, tag="e1", name="e1", bufs=EBUFS)
        e2 = pw.tile([128, 3, 2, 128], u8, tag="e2", name="e2", bufs=EBUFS)
        ev1 = e1[:].rearrange("p ns kt j -> p kt ns j")
        ev2 = e2[:].rearrange("p ns kt j -> p kt ns j")
        nc.scalar.activation(ev1.bitcast(fp8e5)[:, :, 0:a, :],
                             stv[:, :, 0:a, :], AF.Exp, scale=SCALE)
        nc.vector.tensor_scalar(ev2[:, :, 0:4 - a, :], stv[:, :, a:4, :],
                                SCALE * 4.0 * LOG2E, 60.0,
                                op0=ALU.mult, op1=ALU.add)
        return (a, e1, e2)

    def emit_avx(half, pr, e, ots):
        # AVx accumulation + denominator columns (in ots[0] cols 256:260)
        a, e1, e2 = e
        xr = xTw8[:, pr].rearrange("p t kt j -> p kt t j")
        els = [(e1 if ns < a else e2)[:, ns if ns < a else ns - a
                                      ].bitcast(fp8e5) for ns in range(4)]
        for ns in range(4):
            nc.tensor.matmul(
                ots[ns][:, 0:256], els[ns], xr,
                start=(pr == 0), stop=(pr == 15), perf_mode=DR,
            )
        for ns in range(4):
            nc.tensor.matmul(
                ots[0][:, 256 + ns:257 + ns], els[ns], ones5[:],
                start=(pr == 0), stop=(pr == 15), perf_mode=DR,
            )


    def finish_steps(half, ots, use_sp=False):
        # generator of emission steps; each step is interleaved between the
        # next half's pairs so no engine queue sees a serial finish block.
        rec4 = ptiny.tile([128, 4], f32, tag="rec4", name="rec4")
        nc.vector.reciprocal(rec4[:], ots[0][:, 256:260])
        trps = [None, None]
        ons = []

        def emit_on(ns):
            on = pw.tile([128, 256], f32, tag="on", name="on", bufs=5)
            if ON_ACT[half * 4 + ns]:
                nc.scalar.activation(on[:], ots[ns][:, 0:256], AF.Identity,
                                     scale=rec4[:, ns:ns + 1])
            else:
                nc.vector.tensor_scalar_mul(on[:], ots[ns][:, 0:256],
                                            rec4[:, ns:ns + 1])
            ons.append(on)
            if ns == 0:
                trps[0] = ot.tile([128, 512], f32, tag="warm",
                                  name=f"trp{half}_0")[:]
                trps[1] = ot.tile([128, 512], f32, tag="warm",
                                  name=f"trp{half}_1")[:]
            for t in range(2):
                nc.tensor.transpose(trps[t][:, ns * 128:(ns + 1) * 128],
                                    ons[ns][:, t * 128:(t + 1) * 128], ident)

        def emit_drain(t):
            dst = attnx8[:, t, half]
            if TRP_ACT[half * 2 + t]:
                nc.scalar.activation(dst, trps[t], AF.Copy)
            else:
                nc.vector.tensor_copy(dst, trps[t])

        pjs = [None, None]

        def emit_preload(oh):
            # residual preload: exact fp32 identity matmul of xqb*WS into the
            # projection psum; the DR projection then accumulates on top.
            if use_sp:
                if pjs[0] is None:
                    pp = sp.tile([128, 1024], f32, tag="sp", name=f"pj{half}")
                    pjs[0], pjs[1] = pp[:, 0:512], pp[:, 512:1024]
                pj = pjs[oh]
            else:
                pj = ot.tile([128, 512], f32, tag="warm",
                             name=f"pj{half}_{oh}")[:]
                pjs[oh] = pj
            nc.tensor.matmul(pj, identw,
                             xqb[:, oh, half * 512:(half + 1) * 512],
                             start=True, stop=False, skip_group_check=True)

        def emit_proj(oh):
            pj = pjs[oh]
            nc.tensor.matmul(pj, w238[:, oh], attnx8[:, :, half],
                             start=False, stop=True, perf_mode=DR,
                             skip_group_check=True)
            sl = yts[oh][:, half * 512:(half + 1) * 512]
            if (half + oh) % 2 == 0:
                nc.scalar.activation(sl, pj, AF.Copy, scale=1.0 / WS)
            else:
                nc.vector.tensor_scalar_mul(sl, pj, 1.0 / WS)
            dq = (nc.sync, nc.scalar)[(half + oh) % 2]
            dq.dma_start(
                d["y"][:, oh * NSH + half * 512:oh * NSH + (half + 1) * 512],
                sl,
            )

        def emit_proj_stt(oh):
            # tail half: DVE is free -> plain projection + stt keeps the
            # in-order PE stream short.
            pj = ot.tile([128, 512], f32, tag="warm",
                         name=f"pj{half}_{oh}")[:]
            nc.tensor.matmul(pj, w238[:, oh], attnx8[:, :, half],
                             start=True, stop=True, perf_mode=DR)
            sl = yts[oh][:, half * 512:(half + 1) * 512]
            nc.vector.scalar_tensor_tensor(
                sl, pj, 1.0 / WS, xqb[:, oh, half * 512:(half + 1) * 512],
                op0=ALU.mult, op1=ALU.add,
            )
            dq = (nc.sync, nc.scalar)[(half + oh) % 2]
            dq.dma_start(
                d["y"][:, oh * NSH + half * 512:oh * NSH + (half + 1) * 512],
                sl,
            )

        yield lambda: emit_on(0)
        yield lambda: emit_on(1)
        yield lambda: emit_on(2)
        yield lambda: emit_on(3)
        yield lambda: emit_drain(0)
        yield lambda: emit_drain(1)
        if not use_sp:
            yield lambda: emit_preload(0)
            yield lambda: emit_preload(1)
            yield lambda: emit_proj(0)
            yield lambda: emit_proj(1)
        else:
            yield lambda: emit_proj_stt(0)
            yield lambda: emit_proj_stt(1)

    pending = iter([lambda: emit_xqb(0), lambda: emit_xqb(1)])
    for half in range(2):
        ots = [ot.tile([128, 512], f32, tag="warm", name=f"ot{half}_{ns}")
               for ns in range(4)]
        # PE-stream order per pair p: exp(p), S(p+2), AVx(p) -- the next-next
        # S only needs exp(p)'s slot, so it must not sit behind AVx(p) in the
        # in-order PE queue (that chained AVx latency into every exp cycle).
        sts = {0: emit_s(half, 0), 1: emit_s(half, 1)}
        for pr in range(16):
            e_cur = emit_exp(half, pr, sts.pop(pr))
            if pr + 2 < 16:
                sts[pr + 2] = emit_s(half, pr + 2)
            emit_avx(half, pr, e_cur, ots)
            if pending is not None:
                nxt = next(pending, None)
                if nxt is None:
                    pending = None
                else:
                    nxt()
        pending = finish_steps(half, ots, use_sp=(half == 1))
    for step in pending:
        step()

    ctx.close()


_CACHE = {}


def _get_program():
    if "nc" in _CACHE:
        return _CACHE["nc"], _CACHE["dram"]
    nc = bacc.Bacc("TRN2", target_bir_lowering=False, debug=False,
                   enable_asserts=False, num_devices=NCORES)
    d = {}
    d["xs8"] = nc.dram_tensor("xs8", [128, MT * 256], fp8,
                              kind="ExternalInput").ap()
    d["xTw8"] = nc.dram_tensor("xTw8", [128, 16 * 512], fp8,
                               kind="ExternalInput").ap()
    d["xq"] = nc.dram_tensor("xq", [128, 2 * NSH], f32, kind="ExternalInput").ap()
    d["wb"] = nc.dram_tensor("wb", [128, 4 * 128], bf16, kind="ExternalInput").ap()
    d["w1p8"] = nc.dram_tensor("w1p8", [128, 4 * 128], fp8,
                               kind="ExternalInput").ap()
    d["w23t"] = nc.dram_tensor("w23t", [128, 4 * 128], bf16,
                               kind="ExternalInput").ap()
    d["cpack"] = nc.dram_tensor("cpack", [128, CPW], f32,
                                kind="ExternalInput").ap()
    d["y"] = nc.dram_tensor("y", [128, 2 * NSH], f32, kind="ExternalOutput").ap()

    with tile.TileContext(nc) as tc:
        _build_body(nc, tc, d)
    nc.compile()
    _CACHE["nc"] = nc
    _CACHE["dram"] = d
    return nc, d


def make_in_maps(x, gamma, beta, w0, b0, w1, b1, w2, b2, w3, b3):
    """Host-side sharding/layout prep: returns list of 8 per-core inputs."""
    e4 = ml_dtypes.float8_e4m3
    xb = np.ascontiguousarray(np.asarray(x, np.float32).reshape(B, C, N))

    cpack = np.zeros((128, CPW), np.float32)
    gamma = np.asarray(gamma, np.float32)
    beta = np.asarray(beta, np.float32)
    b0 = np.asarray(b0, np.float32)
    bout = (np.asarray(w3, np.float32) @ np.asarray(b2, np.float32)
            + np.asarray(b3, np.float32))
    for t, off in ((0, CV0), (1, CV1)):
        sl = slice(t * 128, (t + 1) * 128)
        cpack[:, off + 0] = gamma[sl]
        cpack[:, off + 1] = beta[sl]
        cpack[:, off + 2] = b0[sl]
        cpack[:, off + 3] = bout[sl]
        cpack[:, off + 4] = -gamma[sl]
    for t, off in ((0, GMA), (1, GMB)):
        ch = t * 128 + np.arange(128)
        cpack[np.arange(128), off + ch // CPG] = 1.0 / NPG
    for t, off in ((0, GTA), (1, GTB)):
        ch = t * 128 + np.arange(128)
        cpack[ch // CPG, off + np.arange(128)] = gamma[ch]
    cpack[:, IDT:IDT + 128] = np.eye(128, dtype=np.float32)
    cpack[:, IDW:IDW + 128] = np.eye(128, dtype=np.float32) * WS
    cpack[:, EPC] = EPS

    # wb[p, oh, kt, j] = w0^T[kt*128+p, oh*128+j]
    w0t = np.asarray(w0, np.float32).T.reshape(2, 128, 2, 128)  # [kt, p, oh, j]
    wb = w0t.transpose(1, 2, 0, 3).reshape(128, -1).astype(ml_dtypes.bfloat16)
    # w1p8[p, cs, kto, j] = w1[kto*128+p, cs*128+j]
    w1a = np.asarray(w1, np.float32).reshape(2, 128, 2, 128)    # [kto, p, cs, j]
    w1p8 = w1a.transpose(1, 2, 0, 3).reshape(128, -1).astype(e4)
    w23 = (np.asarray(w3, np.float32) @ np.asarray(w2, np.float32)).T
    w23t = w23.reshape(2, 128, 2, 128).transpose(1, 2, 0, 3)
    w23t = w23t.reshape(128, -1).astype(ml_dtypes.bfloat16)

    in_maps = []
    for core in range(NCORES):
        b, j = divmod(core, 4)
        # rotate the token blocks so this core's query shard is mt 0..7;
        # attention sums over m, so any consistent xs8/xTw8 order works
        rot = np.roll(np.arange(MT), -j * 8)
        xc = xb[b]
        xs8 = xc.reshape(2, 128, MT, 128).transpose(1, 2, 0, 3)[:, rot]
        xT = xc.reshape(2, 128, 16, 2, 128)  # [t, jj, pr, kt, p]
        xT = xT.transpose(4, 2, 3, 0, 1).reshape(128, MT, 2, 128)
        xT = xT[:, rot].reshape(128, 16, 2, 2, 128).transpose(0, 1, 3, 2, 4)
        xqc = xc[:, j * NSH:(j + 1) * NSH]
        xq = xqc.reshape(2, 128, NSH).transpose(1, 0, 2).reshape(128, -1)
        m = {
            "xs8": xs8.reshape(128, -1).astype(e4),
            "xTw8": np.ascontiguousarray(xT).reshape(128, -1).astype(e4),
            "xq": np.ascontiguousarray(xq),
            "wb": wb, "w1p8": w1p8, "w23t": w23t, "cpack": cpack,
        }
        in_maps.append(m)
    return in_maps


def assemble_output(results):
    out = np.zeros((B, C, N), np.float32)
    for core in range(NCORES):
        b, j = divmod(core, 4)
        y = results[core]["y"].reshape(128, 2, NSH).transpose(1, 0, 2)
        out[b][:, j * NSH:(j + 1) * NSH] = y.reshape(C, NSH)
    return out.reshape(B, C, 16, 16, 16)


def kernel(x, gamma, beta, w0, b0, w1, b1, w2, b2, w3, b3):
    nc, _ = _get_program()
    in_maps = make_in_maps(x, gamma, beta, w0, b0, w1, b1, w2, b2, w3, b3)
    res = bass_utils.run_bass_kernel_spmd(nc, in_maps, core_ids=list(range(NCORES)))
    return assemble_output(res.results)
